# revision 5
# baseline (speedup 1.0000x reference)
"""Trainium2 Bass kernel for a dense transformer decoder layer.

Strategy (8 NeuronCores, SPMD, uniform program):
  - Tokens (flattened batch*seq = 4096) are sharded 512/core for LayerNorm,
    wo-projection, and the FFN.
  - Attention is sharded over heads: core r owns heads {2r, 2r+1} over the
    full sequence (uniform causal block structure on every core).
  - Collective 1: AllGather of the LN1 output, transposed (feature-major).
  - Collective 2: AllToAll converting head-sharded attention output into
    token-sharded full-head attnT (feeds the wo matmul directly as lhsT).
  - All large matmuls run in float32r (TF32-like, ~1.5e-4 relerr, full PE
    rate at N>=256); transposes of fp32 activations use exact fp32 PE
    transpose mode.
"""
import numpy as np

N_CORES = 8
B, S, D, H, E, DFF = 2, 2048, 1024, 16, 64, 4096
T = B * S              # 4096 flat tokens
TOK = T // N_CORES     # 512 tokens per core
P = 128
EPS = 1e-5

_CACHE = {}


def _build(apply_ln):
    from contextlib import ExitStack
    import concourse.bass as bass
    import concourse.tile as tile
    from concourse import bacc, mybir
    from concourse.masks import make_identity

    FP32 = mybir.dt.float32
    FP32R = mybir.dt.float32r
    AF = mybir.ActivationFunctionType
    SUB = mybir.AluOpType.subtract
    MULT = mybir.AluOpType.mult

    nc = bacc.Bacc("TRN2", target_bir_lowering=False, debug=False,
                   num_devices=N_CORES)

    x_c = nc.dram_tensor("x_c", [TOK, D], FP32, kind="ExternalInput").ap()
    wq_c = nc.dram_tensor("wq_c", [D, P], FP32R, kind="ExternalInput").ap()
    wk_c = nc.dram_tensor("wk_c", [D, P], FP32R, kind="ExternalInput").ap()
    wv_c = nc.dram_tensor("wv_c", [D, P], FP32R, kind="ExternalInput").ap()
    wo_d = nc.dram_tensor("wo", [D, D], FP32R, kind="ExternalInput").ap()
    w1t_d = nc.dram_tensor("w1t", [D, DFF], FP32R, kind="ExternalInput").ap()
    w2t_d = nc.dram_tensor("w2t", [DFF, D], FP32R, kind="ExternalInput").ap()
    b1_d = nc.dram_tensor("b1", [DFF], FP32, kind="ExternalInput").ap()
    b2_d = nc.dram_tensor("b2", [D], FP32, kind="ExternalInput").ap()
    masks_d = nc.dram_tensor("masks", [4, P, 512], FP32R,
                             kind="ExternalInput").ap()
    ln_d = {}
    if apply_ln:
        ln_d = {k: nc.dram_tensor(k, [D], FP32, kind="ExternalInput").ap()
                for k in ("ln1_w", "ln1_b", "ln2_w", "ln2_b")}
    out_d = nc.dram_tensor("out", [TOK, D], FP32, kind="ExternalOutput").ap()

    RG = [list(range(N_CORES))]

    with tile.TileContext(nc) as tc, ExitStack() as top:
        const = top.enter_context(tc.tile_pool(name="const", bufs=1))
        resid = top.enter_context(tc.tile_pool(name="resid", bufs=1))
        dram = top.enter_context(tc.tile_pool(name="dram", bufs=1,
                                              space="DRAM"))

        # ---------------- constants ----------------
        ident = const.tile([P, P], FP32, tag="ident")
        make_identity(nc, ident)
        ident_r = const.tile([P, P], FP32R, tag="ident_r")
        nc.vector.tensor_copy(ident_r[:], ident[:])
        ones_f = const.tile([P, 512], FP32, tag="ones_f")
        nc.vector.memset(ones_f[:], 1.0)
        ones_r = const.tile([P, 512], FP32R, tag="ones_r")
        nc.vector.tensor_copy(ones_r[:], ones_f[:])
        eps_t = const.tile([P, 1], FP32, tag="eps")
        nc.vector.memset(eps_t[:], EPS)

        wq_sb = const.tile([P, 8, P], FP32R, tag="wq")
        wk_sb = const.tile([P, 8, P], FP32R, tag="wk")
        wv_sb = const.tile([P, 8, P], FP32R, tag="wv")
        for w_sb, w_ap in ((wq_sb, wq_c), (wk_sb, wk_c), (wv_sb, wv_c)):
            nc.sync.dma_start(
                w_sb[:], w_ap.rearrange("(dc p) m -> p dc m", p=P))

        # b1 laid out [P, 32]: element (p, fc) = b1[fc*128 + p]  (ACT bias)
        b1_sb = const.tile([P, 32], FP32, tag="b1")
        nc.sync.dma_start(b1_sb[:], b1_d.rearrange("(fc p) -> p fc", p=P))
        b2f = const.tile([1, D], FP32, tag="b2f")
        nc.sync.dma_start(b2f[:], b2_d.rearrange("(o f) -> o f", o=1))
        b2r = const.tile([1, D], FP32R, tag="b2r")
        nc.vector.tensor_copy(b2r[:], b2f[:])

        ln_sb = {}
        for k in ln_d:
            lnt = const.tile([P, D], FP32, tag=k, name=f"lnt_{k}")
            src = ln_d[k]
            bcast = bass.AP(tensor=src.tensor, offset=src.offset,
                            ap=[[0, P]] + list(src.ap))
            nc.sync.dma_start(lnt[:], bcast)
            ln_sb[k] = lnt

        def layer_norm(pool, x_tile, tag, which):
            """x_tile [P, D] fp32 -> y [P, D] fp32 (normalized)."""
            xr = x_tile.rearrange("p (g f) -> p g f", g=2)
            stats = pool.tile([P, 2, 6], FP32, bufs=2, tag=tag + "st",
                              name=tag + "st")
            for g in range(2):
                nc.vector.bn_stats(out=stats[:, g, :], in_=xr[:, g, :])
            mv = pool.tile([P, 2], FP32, bufs=2, tag=tag + "mv",
                           name=tag + "mv")
            nc.vector.bn_aggr(out=mv[:], in_=stats[:])
            std = pool.tile([P, 1], FP32, bufs=2, tag=tag + "sd",
                            name=tag + "sd")
            nc.scalar.activation(out=std[:], in_=mv[:, 1:2], func=AF.Sqrt,
                                 bias=eps_t[:])
            rstd = pool.tile([P, 1], FP32, bufs=2, tag=tag + "rs",
                             name=tag + "rs")
            nc.vector.reciprocal(out=rstd[:], in_=std[:])
            y = pool.tile([P, D], FP32, bufs=2, tag=tag + "y", name=tag + "y")
            nc.vector.tensor_scalar(out=y[:], in0=x_tile[:],
                                    scalar1=mv[:, 0:1], scalar2=rstd[:],
                                    op0=SUB, op1=MULT)
            if apply_ln:
                nc.vector.tensor_mul(y[:], y[:], ln_sb[which + "_w"][:])
                nc.vector.tensor_add(y[:], y[:], ln_sb[which + "_b"][:])
            return y

        xt = []
        x1 = []
        for st in range(4):
            xti = resid.tile([P, D], FP32, tag=f"xt{st}", name=f"xt{st}")
            xt.append(xti)
            x1t = resid.tile([P, D], FP32, tag=f"x1{st}", name=f"x1_{st}")
            x1.append(x1t)

        # ---------------- P1: LN1 + transpose ----------------
        with tc.tile_pool(name="p1", bufs=1) as p1, \
             tc.tile_pool(name="ps1", bufs=1, space="PSUM") as ps1:
            yT = p1.tile([P, 8, 512], FP32R, tag="yT")
            for st in range(4):
                nc.sync.dma_start(xt[st][:], x_c[st * P:(st + 1) * P, :])
                y = layer_norm(p1, xt[st], "l1", "ln1")
                for dc in range(8):
                    ptt = ps1.tile([P, P], FP32, bufs=2, tag="pt",
                                   name="ptt")
                    nc.tensor.transpose(ptt[:], y[:, dc * P:(dc + 1) * P],
                                        ident[:])
                    nc.any.tensor_copy(yT[:, dc, st * P:(st + 1) * P],
                                       ptt[:])
            ytc = dram.tile([D, TOK], FP32R, tag="ytc")
            nc.sync.dma_start(ytc.rearrange("(dc p) t -> p dc t", p=P),
                              yT[:])

        # ---------------- P2: AllGather yT ----------------
        ytg = dram.tile([N_CORES * D, TOK], FP32R, tag="ytg")
        nc.gpsimd.collective_compute(
            "AllGather", mybir.AluOpType.bypass, replica_groups=RG,
            ins=[ytc.opt()], outs=[ytg.opt()])

        atc = dram.tile([N_CORES * P, TOK], FP32R, tag="atc")

        with tc.tile_pool(name="pwo", bufs=1) as pwo_pool:
            wo_sb = pwo_pool.tile([P, 8, D], FP32R, tag="wo")
            nc.sync.dma_start(wo_sb[:],
                              wo_d.rearrange("(dc p) n -> p dc n", p=P))

            with tc.tile_pool(name="p3", bufs=1) as p3:
                masks_sb = p3.tile([P, 4, 512], FP32R, tag="masks")
                nc.sync.dma_start(masks_sb[:],
                                  masks_d.rearrange("m p s -> p m s"))
                qT = p3.tile([P, T], FP32R, tag="qT")
                kT = p3.tile([P, T], FP32R, tag="kT")
                vext = p3.tile([P, 32, 130], FP32R, tag="vext")

                # ------------ P3: QKV over full sequence ------------
                with tc.tile_pool(name="ps3", bufs=1, space="PSUM") as ps3:
                    for rb in range(8):
                        yts = []
                        for dc in range(8):
                            yt_t = p3.tile([P, 512], FP32R, bufs=10,
                                           tag="ytg_t", name="yt_t")
                            base = rb * D + dc * P
                            nc.sync.dma_start(yt_t[:],
                                              ytg[base:base + P, :])
                            yts.append(yt_t)
                        cols = slice(rb * 512, (rb + 1) * 512)
                        for w_sb, dst in ((wq_sb, qT), (wk_sb, kT)):
                            pq = ps3.tile([P, 512], FP32, bufs=2, tag="pq",
                                          name="pq")
                            for dc in range(8):
                                nc.tensor.matmul(pq[:], w_sb[:, dc, :],
                                                 yts[dc][:],
                                                 start=(dc == 0),
                                                 stop=(dc == 7))
                            nc.any.tensor_copy(dst[:, cols], pq[:])
                        pv = ps3.tile([P, 512], FP32, bufs=2, tag="pq",
                                      name="pv")
                        for dc in range(8):
                            nc.tensor.matmul(pv[:], wv_sb[:, dc, :],
                                             yts[dc][:], start=(dc == 0),
                                             stop=(dc == 7))
                        vt_tmp = p3.tile([P, 512], FP32R, bufs=2, tag="vtt",
                                         name="vt_tmp")
                        nc.any.tensor_copy(vt_tmp[:], pv[:])
                        for t4 in range(4):
                            tch = rb * 4 + t4
                            pvt = ps3.tile([P, P], FP32R, bufs=2, tag="pvt",
                                           name="pvt")
                            nc.tensor.transpose(
                                pvt[:], vt_tmp[:, t4 * P:(t4 + 1) * P],
                                ident_r[:])
                            nc.any.tensor_copy(vext[:, tch, 0:64],
                                               pvt[:, 0:64])
                            nc.any.tensor_copy(vext[:, tch, 65:129],
                                               pvt[:, 64:128])
                            nc.any.tensor_copy(vext[:, tch, 64:65],
                                               ones_f[:, 0:1])
                            nc.any.tensor_copy(vext[:, tch, 129:130],
                                               ones_f[:, 0:1])

                # ------------ P4: attention ------------
                with tc.tile_pool(name="ps4", bufs=1, space="PSUM") as ps4:
                    for b in range(2):
                        for qb in range(4):
                            j_blk = 4 * b + qb
                            qc = slice(j_blk * 512, (j_blk + 1) * 512)
                            nt = 4 * (qb + 1)
                            pa0 = ps4.tile([65, 512], FP32, bufs=1,
                                           tag="pa0", name="pa0")
                            pa1 = ps4.tile([65, 512], FP32, bufs=1,
                                           tag="pa1", name="pa1")
                            pending = []
                            for j in range(nt):
                                tch = 16 * b + j
                                kc = slice(tch * P, (tch + 1) * P)
                                psc0 = ps4.tile([P, 512], FP32, bufs=2,
                                                tag="sc0", name="psc0")
                                psc1 = ps4.tile([P, 512], FP32, bufs=2,
                                                tag="sc1", name="psc1")
                                nc.tensor.matmul(psc0[:], kT[0:64, kc],
                                                 qT[0:64, qc],
                                                 start=True, stop=True)
                                nc.tensor.matmul(psc1[:], kT[64:128, kc],
                                                 qT[64:128, qc],
                                                 start=True, stop=True)
                                pt0 = p3.tile([P, 512], FP32R, bufs=3,
                                              tag="pt0", name="pt0")
                                pt1 = p3.tile([P, 512], FP32R, bufs=3,
                                              tag="pt1", name="pt1")
                                nc.scalar.activation(out=pt0[:], in_=psc0[:],
                                                     func=AF.Exp,
                                                     scale=0.125)
                                nc.scalar.activation(out=pt1[:], in_=psc1[:],
                                                     func=AF.Exp,
                                                     scale=0.125)
                                if j >= nt - 4:
                                    m = j - (nt - 4)
                                    nc.vector.tensor_mul(pt0[:], pt0[:],
                                                         masks_sb[:, m, :])
                                    nc.vector.tensor_mul(pt1[:], pt1[:],
                                                         masks_sb[:, m, :])
                                pending.append((tch, pt0, pt1, j == 0,
                                                j == nt - 1))
                                if len(pending) == 2:
                                    _emit_attn(nc, vext, pa0, pa1,
                                               pending.pop(0))
                            while pending:
                                _emit_attn(nc, vext, pa0, pa1,
                                           pending.pop(0))

                            for hl, pa in ((0, pa0), (1, pa1)):
                                sa = p3.tile([65, 512], FP32, bufs=2,
                                             tag="sa", name="sa")
                                nc.any.tensor_copy(sa[:], pa[:])
                                rsf = p3.tile([65, 512], FP32, bufs=2,
                                              tag="rsf", name="rsf")
                                nc.vector.reciprocal(out=rsf[64:65, :],
                                                     in_=sa[64:65, :])
                                rsr = p3.tile([65, 512], FP32R, bufs=2,
                                              tag="rsr", name="rsr")
                                nc.vector.tensor_copy(rsr[64:65, :],
                                                      rsf[64:65, :])
                                pb = ps4.tile([64, 512], FP32, bufs=1,
                                              tag="bc", name="pb")
                                nc.tensor.matmul(pb[:], ones_r[64:65, 0:64],
                                                 rsr[64:65, :],
                                                 start=True, stop=True)
                                an = p3.tile([64, 512], FP32R, bufs=2,
                                             tag="an", name="an")
                                nc.vector.tensor_mul(an[:], sa[0:64, :],
                                                     pb[:])
                                row = j_blk * P + hl * 64
                                nc.sync.dma_start(atc[row:row + 64, :],
                                                  an[:])

            # ---------------- P5: AllToAll attnT ----------------
            atg = dram.tile([N_CORES * P, TOK], FP32R, tag="atg")
            nc.gpsimd.collective_compute(
                "AllToAll", mybir.AluOpType.bypass, replica_groups=RG,
                ins=[atc.opt()], outs=[atg.opt()])

            # ---------------- P6: wo + residual ----------------
            with tc.tile_pool(name="p6", bufs=1) as p6, \
                 tc.tile_pool(name="ps6", bufs=1, space="PSUM") as ps6:
                at2 = p6.tile([P, 8, 512], FP32R, tag="at2")
                nc.sync.dma_start(at2[:],
                                  atg.rearrange("(rr p) t -> p rr t", p=P))
                for st in range(4):
                    for ncol in range(2):
                        pw = ps6.tile([P, 512], FP32, bufs=2, tag="pwo",
                                      name="pw")
                        for rr in range(8):
                            nc.tensor.matmul(
                                pw[:], at2[:, rr, st * P:(st + 1) * P],
                                wo_sb[:, rr, ncol * 512:(ncol + 1) * 512],
                                start=(rr == 0), stop=(rr == 7))
                        nc.vector.tensor_add(
                            x1[st][:, ncol * 512:(ncol + 1) * 512], pw[:],
                            xt[st][:, ncol * 512:(ncol + 1) * 512])

        # ---------------- P7: LN2 + transpose ----------------
        with tc.tile_pool(name="p7", bufs=1) as p7:
            with tc.tile_pool(name="ps7", bufs=1, space="PSUM") as ps7:
                y2T = p7.tile([P, 8, 512], FP32R, tag="y2T")
                for st in range(4):
                    y2 = layer_norm(p7, x1[st], "l2", "ln2")
                    for dc in range(8):
                        ptt2 = ps7.tile([P, P], FP32, bufs=2, tag="pt2",
                                        name="ptt2")
                        nc.tensor.transpose(ptt2[:],
                                            y2[:, dc * P:(dc + 1) * P],
                                            ident[:])
                        nc.any.tensor_copy(
                            y2T[:, dc, st * P:(st + 1) * P], ptt2[:])

            # ---------------- P8/P9: FFN ----------------
            with tc.tile_pool(name="p8", bufs=1) as p8, \
                 tc.tile_pool(name="ps8", bufs=1, space="PSUM") as ps8:
                hT = p8.tile([P, 32, 512], FP32R, tag="hT")
                w1t_r = w1t_d.rearrange("(dc p) (fc m) -> p dc fc m",
                                        p=P, m=P)
                for fc in range(32):
                    w1tt = p8.tile([P, 8, P], FP32R, bufs=4, tag="w1tt",
                                   name="w1tt")
                    nc.sync.dma_start(w1tt[:], w1t_r[:, :, fc, :])
                    ph = ps8.tile([P, 512], FP32, bufs=2, tag="ph",
                                  name="ph")
                    for dc in range(8):
                        nc.tensor.matmul(ph[:], w1tt[:, dc, :],
                                         y2T[:, dc, :], start=(dc == 0),
                                         stop=(dc == 7))
                    nc.scalar.activation(out=hT[:, fc, :], in_=ph[:],
                                         func=AF.Gelu_apprx_tanh,
                                         bias=b1_sb[:, fc:fc + 1])
                for ncol in range(2):
                    nc2 = slice(ncol * 512, (ncol + 1) * 512)
                    po = [ps8.tile([P, 512], FP32, bufs=1, tag=f"po{sc}",
                                   name=f"po_{ncol}_{sc}")
                          for sc in range(4)]
                    for sc in range(4):
                        nc.tensor.matmul(po[sc][:], ones_r[0:1, 0:128],
                                         b2r[0:1, nc2], start=True,
                                         stop=False)
                    for fc in range(32):
                        w2tt = p8.tile([P, 512], FP32R, bufs=4, tag="w2tt",
                                       name="w2tt")
                        nc.sync.dma_start(w2tt[:],
                                          w2t_d[fc * P:(fc + 1) * P, nc2])
                        for sc in range(4):
                            nc.tensor.matmul(
                                po[sc][:], hT[:, fc, sc * P:(sc + 1) * P],
                                w2tt[:], start=False, stop=(fc == 31))
                    for sc in range(4):
                        oh = p8.tile([P, 512], FP32, bufs=2, tag="oh",
                                     name="oh")
                        nc.vector.tensor_add(oh[:], po[sc][:],
                                             x1[sc][:, nc2])
                        nc.sync.dma_start(out_d[sc * P:(sc + 1) * P, nc2],
                                          oh[:])

    nc.compile()
    return nc


def _emit_attn(nc, vext, pa0, pa1, item):
    tch, pt0, pt1, is_first, is_last = item
    nc.tensor.matmul(pa0[:], vext[:, tch, 0:65], pt0[:],
                     start=is_first, stop=is_last)
    nc.tensor.matmul(pa1[:], vext[:, tch, 65:130], pt1[:],
                     start=is_first, stop=is_last)


def _get_nc(apply_ln):
    key = ("nc", apply_ln)
    if key not in _CACHE:
        _CACHE[key] = _build(apply_ln)
    return _CACHE[key]


def _make_masks():
    tt = np.arange(P)[:, None]
    ss = np.arange(512)[None, :]
    return np.stack([(P * m + tt <= ss) for m in range(4)]
                    ).astype(np.float32)


def _prepare(inputs):
    x = np.asarray(inputs["x"], dtype=np.float32).reshape(T, D)
    wq = np.asarray(inputs["wq"], dtype=np.float32)
    wk = np.asarray(inputs["wk"], dtype=np.float32)
    wv = np.asarray(inputs["wv"], dtype=np.float32)
    wo = np.ascontiguousarray(np.asarray(inputs["wo"], dtype=np.float32))
    w1t = np.ascontiguousarray(
        np.asarray(inputs["w1"], dtype=np.float32).T)          # [D, DFF]
    w2t = np.ascontiguousarray(
        np.asarray(inputs["w2"], dtype=np.float32).T)          # [DFF, D]
    b1 = np.asarray(inputs["b1"], dtype=np.float32)
    b2 = np.asarray(inputs["b2"], dtype=np.float32)
    masks = _make_masks()

    apply_ln = not (
        np.all(np.asarray(inputs["ln1_w"]) == 1)
        and np.all(np.asarray(inputs["ln1_b"]) == 0)
        and np.all(np.asarray(inputs["ln2_w"]) == 1)
        and np.all(np.asarray(inputs["ln2_b"]) == 0))

    in_maps = []
    for r in range(N_CORES):
        m = {
            "x_c": np.ascontiguousarray(x[r * TOK:(r + 1) * TOK]),
            "wq_c": np.ascontiguousarray(
                np.concatenate([wq[2 * r], wq[2 * r + 1]], axis=1)),
            "wk_c": np.ascontiguousarray(
                np.concatenate([wk[2 * r], wk[2 * r + 1]], axis=1)),
            "wv_c": np.ascontiguousarray(
                np.concatenate([wv[2 * r], wv[2 * r + 1]], axis=1)),
            "wo": wo, "w1t": w1t, "w2t": w2t, "b1": b1, "b2": b2,
            "masks": masks,
        }
        if apply_ln:
            for k in ("ln1_w", "ln1_b", "ln2_w", "ln2_b"):
                m[k] = np.asarray(inputs[k], dtype=np.float32)
        in_maps.append(m)
    return in_maps, apply_ln


def _run(inputs, trace=False):
    from concourse.bass_utils import run_bass_kernel_spmd
    in_maps, apply_ln = _prepare(inputs)
    nc = _get_nc(apply_ln)
    res = run_bass_kernel_spmd(nc, in_maps, list(range(N_CORES)),
                               trace=trace)
    out = np.concatenate([res.results[r]["out"] for r in range(N_CORES)],
                         axis=0).reshape(B, S, D).astype(np.float32)
    return out, res


def kernel(**inputs):
    out, _ = _run(inputs)
    return out


def bench(**inputs):
    """Like kernel() but with NTFF tracing; returns (out, exec_time_ns)."""
    out, res = _run(inputs, trace=True)
    return out, res.exec_time_ns


# revision 7
# speedup vs baseline: 1.2551x; 1.2551x over previous
"""Trainium2 Bass kernel for a dense transformer decoder layer.

Strategy (8 NeuronCores, SPMD, uniform program):
  - Tokens (flattened batch*seq = 4096) are sharded 512/core for LayerNorm,
    wo-projection, and the FFN.
  - Attention is sharded over heads: core r owns heads {2r, 2r+1} over the
    full sequence (uniform causal block structure on every core).
  - Collective 1: AllGather of the LN1 output, transposed (feature-major).
  - Collective 2: AllToAll converting head-sharded attention output into
    token-sharded full-head attnT (feeds the wo matmul directly as lhsT).
  - Matmul operands are bf16 (fast weight load, overlappable LDWEIGHTS);
    all accumulation, LayerNorm, softmax and residual math stays fp32 in
    PSUM/SBUF.
"""
import numpy as np

N_CORES = 8
B, S, D, H, E, DFF = 2, 2048, 1024, 16, 64, 4096
T = B * S              # 4096 flat tokens
TOK = T // N_CORES     # 512 tokens per core
P = 128
EPS = 1e-5

_CACHE = {}


def _build(apply_ln):
    from contextlib import ExitStack
    import concourse.bass as bass
    import concourse.tile as tile
    from concourse import bacc, mybir
    from concourse.masks import make_identity

    FP32 = mybir.dt.float32
    BF16 = mybir.dt.bfloat16
    AF = mybir.ActivationFunctionType
    SUB = mybir.AluOpType.subtract
    MULT = mybir.AluOpType.mult

    nc = bacc.Bacc("TRN2", target_bir_lowering=False, debug=False,
                   num_devices=N_CORES)

    x_c = nc.dram_tensor("x_c", [TOK, D], FP32, kind="ExternalInput").ap()
    wq_c = nc.dram_tensor("wq_c", [D, P], BF16, kind="ExternalInput").ap()
    wk_c = nc.dram_tensor("wk_c", [D, P], BF16, kind="ExternalInput").ap()
    wv_c = nc.dram_tensor("wv_c", [D, P], BF16, kind="ExternalInput").ap()
    wo_d = nc.dram_tensor("wo", [D, D], BF16, kind="ExternalInput").ap()
    w1t_d = nc.dram_tensor("w1t", [D, DFF], BF16, kind="ExternalInput").ap()
    w2t_d = nc.dram_tensor("w2t", [DFF, D], BF16, kind="ExternalInput").ap()
    b1_d = nc.dram_tensor("b1", [DFF], FP32, kind="ExternalInput").ap()
    b2_d = nc.dram_tensor("b2", [D], FP32, kind="ExternalInput").ap()
    masks_d = nc.dram_tensor("masks", [4, P, 512], BF16,
                             kind="ExternalInput").ap()
    ln_d = {}
    if apply_ln:
        ln_d = {k: nc.dram_tensor(k, [D], FP32, kind="ExternalInput").ap()
                for k in ("ln1_w", "ln1_b", "ln2_w", "ln2_b")}
    out_d = nc.dram_tensor("out", [TOK, D], FP32, kind="ExternalOutput").ap()

    RG = [list(range(N_CORES))]

    with tile.TileContext(nc) as tc, ExitStack() as top:
        const = top.enter_context(tc.tile_pool(name="const", bufs=1))
        resid = top.enter_context(tc.tile_pool(name="resid", bufs=1))
        dram = top.enter_context(tc.tile_pool(name="dram", bufs=1,
                                              space="DRAM"))

        # ---------------- constants ----------------
        ident_f = const.tile([P, P], FP32, tag="ident_f")
        make_identity(nc, ident_f)
        ident = const.tile([P, P], BF16, tag="ident")
        nc.vector.tensor_copy(ident[:], ident_f[:])
        ones_f = const.tile([P, 128], FP32, tag="ones_f")
        nc.vector.memset(ones_f[:], 1.0)
        ones_b = const.tile([P, 128], BF16, tag="ones_b")
        nc.vector.tensor_copy(ones_b[:], ones_f[:])
        eps_t = const.tile([P, 1], FP32, tag="eps")
        nc.vector.memset(eps_t[:], EPS)

        wq_sb = const.tile([P, 8, P], BF16, tag="wq")
        wk_sb = const.tile([P, 8, P], BF16, tag="wk")
        wv_sb = const.tile([P, 8, P], BF16, tag="wv")
        for w_sb, w_ap in ((wq_sb, wq_c), (wk_sb, wk_c), (wv_sb, wv_c)):
            nc.sync.dma_start(
                w_sb[:], w_ap.rearrange("(dc p) m -> p dc m", p=P))

        # b1 laid out [P, 32]: element (p, fc) = b1[fc*128 + p]  (ACT bias)
        b1_sb = const.tile([P, 32], FP32, tag="b1")
        nc.sync.dma_start(b1_sb[:], b1_d.rearrange("(fc p) -> p fc", p=P))
        b2f = const.tile([1, D], FP32, tag="b2f")
        nc.sync.dma_start(b2f[:], b2_d.rearrange("(o f) -> o f", o=1))
        b2b = const.tile([1, D], BF16, tag="b2b")
        nc.vector.tensor_copy(b2b[:], b2f[:])

        ln_sb = {}
        for k in ln_d:
            lnt = const.tile([P, D], FP32, tag=k, name=f"lnt_{k}")
            src = ln_d[k]
            bcast = bass.AP(tensor=src.tensor, offset=src.offset,
                            ap=[[0, P]] + list(src.ap))
            nc.sync.dma_start(lnt[:], bcast)
            ln_sb[k] = lnt

        def layer_norm(pool, x_tile, tag, which):
            """x_tile [P, D] fp32 -> y [P, D] bf16 (normalized)."""
            xr = x_tile.rearrange("p (g f) -> p g f", g=2)
            stats = pool.tile([P, 2, 6], FP32, bufs=2, tag=tag + "st",
                              name=tag + "st")
            for g in range(2):
                nc.vector.bn_stats(out=stats[:, g, :], in_=xr[:, g, :])
            mv = pool.tile([P, 2], FP32, bufs=2, tag=tag + "mv",
                           name=tag + "mv")
            nc.vector.bn_aggr(out=mv[:], in_=stats[:])
            std = pool.tile([P, 1], FP32, bufs=2, tag=tag + "sd",
                            name=tag + "sd")
            nc.scalar.activation(out=std[:], in_=mv[:, 1:2], func=AF.Sqrt,
                                 bias=eps_t[:])
            rstd = pool.tile([P, 1], FP32, bufs=2, tag=tag + "rs",
                             name=tag + "rs")
            nc.vector.reciprocal(out=rstd[:], in_=std[:])
            if apply_ln:
                yf = pool.tile([P, D], FP32, bufs=2, tag=tag + "yf",
                               name=tag + "yf")
                nc.vector.tensor_scalar(out=yf[:], in0=x_tile[:],
                                        scalar1=mv[:, 0:1], scalar2=rstd[:],
                                        op0=SUB, op1=MULT)
                nc.vector.tensor_mul(yf[:], yf[:], ln_sb[which + "_w"][:])
                nc.vector.tensor_add(yf[:], yf[:], ln_sb[which + "_b"][:])
                y = pool.tile([P, D], BF16, bufs=2, tag=tag + "y",
                              name=tag + "y")
                nc.vector.tensor_copy(y[:], yf[:])
            else:
                y = pool.tile([P, D], BF16, bufs=2, tag=tag + "y",
                              name=tag + "y")
                nc.vector.tensor_scalar(out=y[:], in0=x_tile[:],
                                        scalar1=mv[:, 0:1], scalar2=rstd[:],
                                        op0=SUB, op1=MULT)
            return y

        xt = []
        x1 = []
        for st in range(4):
            xti = resid.tile([P, D], FP32, tag=f"xt{st}", name=f"xt{st}")
            xt.append(xti)
            x1t = resid.tile([P, D], FP32, tag=f"x1{st}", name=f"x1_{st}")
            x1.append(x1t)

        # ---------------- P1: LN1 + transpose ----------------
        with tc.tile_pool(name="p1", bufs=1) as p1, \
             tc.tile_pool(name="ps1", bufs=1, space="PSUM") as ps1:
            yT = p1.tile([P, 8, 512], BF16, tag="yT")
            for st in range(4):
                nc.sync.dma_start(xt[st][:], x_c[st * P:(st + 1) * P, :])
                y = layer_norm(p1, xt[st], "l1", "ln1")
                for dc in range(8):
                    ptt = ps1.tile([P, P], BF16, bufs=2, tag="pt",
                                   name="ptt")
                    nc.tensor.transpose(ptt[:], y[:, dc * P:(dc + 1) * P],
                                        ident[:])
                    nc.vector.tensor_copy(yT[:, dc, st * P:(st + 1) * P],
                                          ptt[:])
            ytc = dram.tile([D, TOK], BF16, tag="ytc")
            nc.sync.dma_start(ytc.rearrange("(dc p) t -> p dc t", p=P),
                              yT[:])

        # ---------------- P2: AllGather yT ----------------
        ytg = dram.tile([N_CORES * D, TOK], BF16, tag="ytg")
        nc.gpsimd.collective_compute(
            "AllGather", mybir.AluOpType.bypass, replica_groups=RG,
            ins=[ytc.opt()], outs=[ytg.opt()])

        atc = dram.tile([N_CORES * P, TOK], BF16, tag="atc")

        with tc.tile_pool(name="pwo", bufs=1) as pwo_pool:
            wo_sb = pwo_pool.tile([P, 8, D], BF16, tag="wo")
            nc.sync.dma_start(wo_sb[:],
                              wo_d.rearrange("(dc p) n -> p dc n", p=P))

            with tc.tile_pool(name="p3", bufs=1) as p3:
                masks_sb = p3.tile([P, 4, 512], BF16, tag="masks")
                nc.sync.dma_start(masks_sb[:],
                                  masks_d.rearrange("m p s -> p m s"))
                qT = p3.tile([P, T], BF16, tag="qT")
                kT = p3.tile([P, T], BF16, tag="kT")
                vext = p3.tile([P, 32, 130], BF16, tag="vext")

                # ------------ P3: QKV over full sequence ------------
                with tc.tile_pool(name="ps3", bufs=1, space="PSUM") as ps3:
                    for rb in range(8):
                        yts = []
                        for dc in range(8):
                            yt_t = p3.tile([P, 512], BF16, bufs=10,
                                           tag="ytg_t", name="yt_t")
                            base = rb * D + dc * P
                            nc.sync.dma_start(yt_t[:],
                                              ytg[base:base + P, :])
                            yts.append(yt_t)
                        cols = slice(rb * 512, (rb + 1) * 512)
                        for w_sb, dst in ((wq_sb, qT), (wk_sb, kT)):
                            pq = ps3.tile([P, 512], FP32, bufs=2, tag="pq",
                                          name="pq")
                            for dc in range(8):
                                nc.tensor.matmul(pq[:], w_sb[:, dc, :],
                                                 yts[dc][:],
                                                 start=(dc == 0),
                                                 stop=(dc == 7))
                            nc.vector.tensor_copy(dst[:, cols], pq[:])
                        pv = ps3.tile([P, 512], FP32, bufs=2, tag="pq",
                                      name="pv")
                        for dc in range(8):
                            nc.tensor.matmul(pv[:], wv_sb[:, dc, :],
                                             yts[dc][:], start=(dc == 0),
                                             stop=(dc == 7))
                        vt_tmp = p3.tile([P, 512], BF16, bufs=2, tag="vtt",
                                         name="vt_tmp")
                        nc.vector.tensor_copy(vt_tmp[:], pv[:])
                        for t4 in range(4):
                            tch = rb * 4 + t4
                            pvt = ps3.tile([P, P], BF16, bufs=2, tag="pvt",
                                           name="pvt")
                            nc.tensor.transpose(
                                pvt[:], vt_tmp[:, t4 * P:(t4 + 1) * P],
                                ident[:])
                            nc.vector.tensor_copy(vext[:, tch, 0:64],
                                                  pvt[:, 0:64])
                            nc.vector.tensor_copy(vext[:, tch, 65:129],
                                                  pvt[:, 64:128])
                            nc.vector.tensor_copy(vext[:, tch, 64:65],
                                                  ones_b[:, 0:1])
                            nc.vector.tensor_copy(vext[:, tch, 129:130],
                                                  ones_b[:, 0:1])

                # ------------ P4: attention ------------
                with tc.tile_pool(name="ps4", bufs=1, space="PSUM") as ps4:
                    for b in range(2):
                        for qb in range(4):
                            j_blk = 4 * b + qb
                            qc = slice(j_blk * 512, (j_blk + 1) * 512)
                            nt = 4 * (qb + 1)
                            pa0 = ps4.tile([65, 512], FP32, bufs=1,
                                           tag="pa0", name="pa0")
                            pa1 = ps4.tile([65, 512], FP32, bufs=1,
                                           tag="pa1", name="pa1")
                            pending = []
                            for j in range(nt):
                                tch = 16 * b + j
                                kc = slice(tch * P, (tch + 1) * P)
                                psc0 = ps4.tile([P, 512], FP32, bufs=2,
                                                tag="sc0", name="psc0")
                                psc1 = ps4.tile([P, 512], FP32, bufs=2,
                                                tag="sc1", name="psc1")
                                nc.tensor.matmul(psc0[:], kT[0:64, kc],
                                                 qT[0:64, qc],
                                                 start=True, stop=True)
                                nc.tensor.matmul(psc1[:], kT[64:128, kc],
                                                 qT[64:128, qc],
                                                 start=True, stop=True)
                                pt0 = p3.tile([P, 512], BF16, bufs=3,
                                              tag="pt0", name="pt0")
                                pt1 = p3.tile([P, 512], BF16, bufs=3,
                                              tag="pt1", name="pt1")
                                nc.scalar.activation(out=pt0[:], in_=psc0[:],
                                                     func=AF.Exp,
                                                     scale=0.125)
                                nc.scalar.activation(out=pt1[:], in_=psc1[:],
                                                     func=AF.Exp,
                                                     scale=0.125)
                                if j >= nt - 4:
                                    m = j - (nt - 4)
                                    nc.vector.tensor_mul(pt0[:], pt0[:],
                                                         masks_sb[:, m, :])
                                    nc.vector.tensor_mul(pt1[:], pt1[:],
                                                         masks_sb[:, m, :])
                                pending.append((tch, pt0, pt1, j == 0,
                                                j == nt - 1))
                                if len(pending) == 2:
                                    _emit_attn(nc, vext, pa0, pa1,
                                               pending.pop(0))
                            while pending:
                                _emit_attn(nc, vext, pa0, pa1,
                                           pending.pop(0))

                            for hl, pa in ((0, pa0), (1, pa1)):
                                sa = p3.tile([65, 512], FP32, bufs=2,
                                             tag="sa", name="sa")
                                nc.vector.tensor_copy(sa[:], pa[:])
                                rsf = p3.tile([65, 512], FP32, bufs=2,
                                              tag="rsf", name="rsf")
                                nc.vector.reciprocal(out=rsf[64:65, :],
                                                     in_=sa[64:65, :])
                                rsr = p3.tile([65, 512], BF16, bufs=2,
                                              tag="rsr", name="rsr")
                                nc.vector.tensor_copy(rsr[64:65, :],
                                                      rsf[64:65, :])
                                pb = ps4.tile([64, 512], FP32, bufs=1,
                                              tag="bc", name="pb")
                                nc.tensor.matmul(pb[:], ones_b[64:65, 0:64],
                                                 rsr[64:65, :],
                                                 start=True, stop=True)
                                an = p3.tile([64, 512], BF16, bufs=2,
                                             tag="an", name="an")
                                nc.vector.tensor_mul(an[:], sa[0:64, :],
                                                     pb[:])
                                row = j_blk * P + hl * 64
                                nc.sync.dma_start(atc[row:row + 64, :],
                                                  an[:])

            # ---------------- P5: AllToAll attnT ----------------
            atg = dram.tile([N_CORES * P, TOK], BF16, tag="atg")
            nc.gpsimd.collective_compute(
                "AllToAll", mybir.AluOpType.bypass, replica_groups=RG,
                ins=[atc.opt()], outs=[atg.opt()])

            # ---------------- P6: wo + residual ----------------
            with tc.tile_pool(name="p6", bufs=1) as p6, \
                 tc.tile_pool(name="ps6", bufs=1, space="PSUM") as ps6:
                at2 = p6.tile([P, 8, 512], BF16, tag="at2")
                nc.sync.dma_start(at2[:],
                                  atg.rearrange("(rr p) t -> p rr t", p=P))
                for st in range(4):
                    for ncol in range(2):
                        pw = ps6.tile([P, 512], FP32, bufs=2, tag="pwo",
                                      name="pw")
                        for rr in range(8):
                            nc.tensor.matmul(
                                pw[:], at2[:, rr, st * P:(st + 1) * P],
                                wo_sb[:, rr, ncol * 512:(ncol + 1) * 512],
                                start=(rr == 0), stop=(rr == 7))
                        nc.vector.tensor_add(
                            x1[st][:, ncol * 512:(ncol + 1) * 512], pw[:],
                            xt[st][:, ncol * 512:(ncol + 1) * 512])

        # ---------------- P7: LN2 + transpose ----------------
        with tc.tile_pool(name="p7", bufs=1) as p7:
            with tc.tile_pool(name="ps7", bufs=1, space="PSUM") as ps7:
                y2T = p7.tile([P, 8, 512], BF16, tag="y2T")
                for st in range(4):
                    y2 = layer_norm(p7, x1[st], "l2", "ln2")
                    for dc in range(8):
                        ptt2 = ps7.tile([P, P], BF16, bufs=2, tag="pt2",
                                        name="ptt2")
                        nc.tensor.transpose(ptt2[:],
                                            y2[:, dc * P:(dc + 1) * P],
                                            ident[:])
                        nc.vector.tensor_copy(
                            y2T[:, dc, st * P:(st + 1) * P], ptt2[:])

            # ---------------- P8/P9: FFN ----------------
            with tc.tile_pool(name="p8", bufs=1) as p8, \
                 tc.tile_pool(name="ps8", bufs=1, space="PSUM") as ps8:
                hT = p8.tile([P, 32, 512], BF16, tag="hT")
                w1t_r = w1t_d.rearrange("(dc p) (fc m) -> p dc fc m",
                                        p=P, m=P)
                for fc in range(32):
                    w1tt = p8.tile([P, 8, P], BF16, bufs=4, tag="w1tt",
                                   name="w1tt")
                    nc.sync.dma_start(w1tt[:], w1t_r[:, :, fc, :])
                    ph = ps8.tile([P, 512], FP32, bufs=2, tag="ph",
                                  name="ph")
                    for dc in range(8):
                        nc.tensor.matmul(ph[:], w1tt[:, dc, :],
                                         y2T[:, dc, :], start=(dc == 0),
                                         stop=(dc == 7))
                    nc.scalar.activation(out=hT[:, fc, :], in_=ph[:],
                                         func=AF.Gelu_apprx_tanh,
                                         bias=b1_sb[:, fc:fc + 1])
                for ncol in range(2):
                    nc2 = slice(ncol * 512, (ncol + 1) * 512)
                    po = [ps8.tile([P, 512], FP32, bufs=1, tag=f"po{sc}",
                                   name=f"po_{ncol}_{sc}")
                          for sc in range(4)]
                    for sc in range(4):
                        nc.tensor.matmul(po[sc][:], ones_b[0:1, :],
                                         b2b[0:1, nc2], start=True,
                                         stop=False)
                    for fc in range(32):
                        w2tt = p8.tile([P, 512], BF16, bufs=4, tag="w2tt",
                                       name="w2tt")
                        nc.sync.dma_start(w2tt[:],
                                          w2t_d[fc * P:(fc + 1) * P, nc2])
                        for sc in range(4):
                            nc.tensor.matmul(
                                po[sc][:], hT[:, fc, sc * P:(sc + 1) * P],
                                w2tt[:], start=False, stop=(fc == 31))
                    for sc in range(4):
                        oh = p8.tile([P, 512], FP32, bufs=2, tag="oh",
                                     name="oh")
                        nc.vector.tensor_add(oh[:], po[sc][:],
                                             x1[sc][:, nc2])
                        nc.sync.dma_start(out_d[sc * P:(sc + 1) * P, nc2],
                                          oh[:])

    nc.compile()
    return nc


def _emit_attn(nc, vext, pa0, pa1, item):
    tch, pt0, pt1, is_first, is_last = item
    nc.tensor.matmul(pa0[:], vext[:, tch, 0:65], pt0[:],
                     start=is_first, stop=is_last)
    nc.tensor.matmul(pa1[:], vext[:, tch, 65:130], pt1[:],
                     start=is_first, stop=is_last)


def _get_nc(apply_ln):
    key = ("nc_v2", apply_ln)
    if key not in _CACHE:
        _CACHE[key] = _build(apply_ln)
    return _CACHE[key]


def _make_masks():
    tt = np.arange(P)[:, None]
    ss = np.arange(512)[None, :]
    return np.stack([(P * m + tt <= ss) for m in range(4)]
                    ).astype(np.float32)


def _bf16(a):
    import ml_dtypes
    return np.asarray(a, dtype=np.float32).astype(ml_dtypes.bfloat16)


def _prepare(inputs):
    x = np.asarray(inputs["x"], dtype=np.float32).reshape(T, D)
    wq = np.asarray(inputs["wq"], dtype=np.float32)
    wk = np.asarray(inputs["wk"], dtype=np.float32)
    wv = np.asarray(inputs["wv"], dtype=np.float32)
    wo = _bf16(inputs["wo"])
    w1t = _bf16(np.asarray(inputs["w1"], dtype=np.float32).T)   # [D, DFF]
    w2t = _bf16(np.asarray(inputs["w2"], dtype=np.float32).T)   # [DFF, D]
    b1 = np.asarray(inputs["b1"], dtype=np.float32)
    b2 = np.asarray(inputs["b2"], dtype=np.float32)
    masks = _bf16(_make_masks())

    apply_ln = not (
        np.all(np.asarray(inputs["ln1_w"]) == 1)
        and np.all(np.asarray(inputs["ln1_b"]) == 0)
        and np.all(np.asarray(inputs["ln2_w"]) == 1)
        and np.all(np.asarray(inputs["ln2_b"]) == 0))

    in_maps = []
    for r in range(N_CORES):
        m = {
            "x_c": np.ascontiguousarray(x[r * TOK:(r + 1) * TOK]),
            "wq_c": _bf16(np.concatenate([wq[2 * r], wq[2 * r + 1]],
                                         axis=1)),
            "wk_c": _bf16(np.concatenate([wk[2 * r], wk[2 * r + 1]],
                                         axis=1)),
            "wv_c": _bf16(np.concatenate([wv[2 * r], wv[2 * r + 1]],
                                         axis=1)),
            "wo": wo, "w1t": w1t, "w2t": w2t, "b1": b1, "b2": b2,
            "masks": masks,
        }
        if apply_ln:
            for k in ("ln1_w", "ln1_b", "ln2_w", "ln2_b"):
                m[k] = np.asarray(inputs[k], dtype=np.float32)
        in_maps.append(m)
    return in_maps, apply_ln


def _run(inputs, trace=False):
    from concourse.bass_utils import run_bass_kernel_spmd
    in_maps, apply_ln = _prepare(inputs)
    nc = _get_nc(apply_ln)
    res = run_bass_kernel_spmd(nc, in_maps, list(range(N_CORES)),
                               trace=trace)
    out = np.concatenate([res.results[r]["out"] for r in range(N_CORES)],
                         axis=0).reshape(B, S, D).astype(np.float32)
    return out, res


def kernel(**inputs):
    out, _ = _run(inputs)
    return out


def bench(**inputs):
    """Like kernel() but with NTFF tracing; returns (out, exec_time_ns)."""
    out, res = _run(inputs, trace=True)
    return out, res.exec_time_ns


# revision 11
# speedup vs baseline: 1.2757x; 1.0164x over previous
"""Trainium2 Bass kernel for a dense transformer decoder layer.

Strategy (8 NeuronCores, SPMD, uniform program):
  - Tokens (flattened batch*seq = 4096) are sharded 512/core for LayerNorm,
    wo-projection, and the FFN.
  - Attention is sharded over heads: core r owns heads {2r, 2r+1} over the
    full sequence (uniform causal block structure on every core).
  - Collective 1: AllGather (split in two halves, overlapped with QKV) of
    the LN1 output, transposed (feature-major).
  - Collective 2: AllToAll converting head-sharded attention output into
    token-sharded full-head attnT (feeds the wo matmul directly as lhsT).
  - Matmul operands are bf16; accumulation, LayerNorm, softmax and
    residual math stays fp32 in PSUM/SBUF.
"""
import numpy as np

N_CORES = 8
B, S, D, H, E, DFF = 2, 2048, 1024, 16, 64, 4096
T = B * S              # 4096 flat tokens
TOK = T // N_CORES     # 512 tokens per core
P = 128
EPS = 1e-5

_CACHE = {}


def _build(apply_ln):
    from contextlib import ExitStack
    import concourse.bass as bass
    import concourse.tile as tile
    from concourse import bacc, mybir
    from concourse.masks import make_identity

    FP32 = mybir.dt.float32
    BF16 = mybir.dt.bfloat16
    AF = mybir.ActivationFunctionType
    SUB = mybir.AluOpType.subtract
    MULT = mybir.AluOpType.mult

    nc = bacc.Bacc("TRN2", target_bir_lowering=False, debug=False,
                   num_devices=N_CORES)

    x_c = nc.dram_tensor("x_c", [TOK, D], FP32, kind="ExternalInput").ap()
    wq_c = nc.dram_tensor("wq_c", [D, P], BF16, kind="ExternalInput").ap()
    wk_c = nc.dram_tensor("wk_c", [D, P], BF16, kind="ExternalInput").ap()
    wv_c = nc.dram_tensor("wv_c", [D, P], BF16, kind="ExternalInput").ap()
    wo_d = nc.dram_tensor("wo", [D, D], BF16, kind="ExternalInput").ap()
    # w1tile[fc, p, dc*128+m] = w1[fc*128+m, dc*128+p]
    w1t_d = nc.dram_tensor("w1tile", [32, P, D], BF16,
                           kind="ExternalInput").ap()
    w2t_d = nc.dram_tensor("w2t", [DFF, D], BF16, kind="ExternalInput").ap()
    b1_d = nc.dram_tensor("b1", [DFF], FP32, kind="ExternalInput").ap()
    b2_d = nc.dram_tensor("b2", [D], FP32, kind="ExternalInput").ap()
    masks_d = nc.dram_tensor("masks", [4, P, 512], BF16,
                             kind="ExternalInput").ap()
    ln_d = {}
    if apply_ln:
        ln_d = {k: nc.dram_tensor(k, [D], FP32, kind="ExternalInput").ap()
                for k in ("ln1_w", "ln1_b", "ln2_w", "ln2_b")}
    out_d = nc.dram_tensor("out", [TOK, D], FP32, kind="ExternalOutput").ap()

    RG = [list(range(N_CORES))]

    with tile.TileContext(nc) as tc, ExitStack() as top:
        const = top.enter_context(tc.tile_pool(name="const", bufs=1))
        resid = top.enter_context(tc.tile_pool(name="resid", bufs=1))
        dram = top.enter_context(tc.tile_pool(name="dram", bufs=1,
                                              space="DRAM"))

        # ---------------- constants ----------------
        ident_f = const.tile([P, P], FP32, tag="ident_f")
        make_identity(nc, ident_f)
        ident = const.tile([P, P], BF16, tag="ident")
        nc.vector.tensor_copy(ident[:], ident_f[:])
        ones_f = const.tile([P, 128], FP32, tag="ones_f")
        nc.vector.memset(ones_f[:], 1.0)
        ones_b = const.tile([P, 128], BF16, tag="ones_b")
        nc.vector.tensor_copy(ones_b[:], ones_f[:])
        eps_t = const.tile([P, 1], FP32, tag="eps")
        nc.vector.memset(eps_t[:], EPS)

        wq_sb = const.tile([P, 8, P], BF16, tag="wq")
        wk_sb = const.tile([P, 8, P], BF16, tag="wk")
        wv_sb = const.tile([P, 8, P], BF16, tag="wv")
        for w_sb, w_ap in ((wq_sb, wq_c), (wk_sb, wk_c), (wv_sb, wv_c)):
            nc.sync.dma_start(
                w_sb[:], w_ap.rearrange("(dc p) m -> p dc m", p=P))

        # b1 laid out [P, 32]: element (p, fc) = b1[fc*128 + p]  (ACT bias)
        b1_sb = const.tile([P, 32], FP32, tag="b1")
        nc.sync.dma_start(b1_sb[:], b1_d.rearrange("(fc p) -> p fc", p=P))
        b2f = const.tile([1, D], FP32, tag="b2f")
        nc.sync.dma_start(b2f[:], b2_d.rearrange("(o f) -> o f", o=1))
        b2b = const.tile([1, D], BF16, tag="b2b")
        nc.vector.tensor_copy(b2b[:], b2f[:])

        ln_sb = {}
        for k in ln_d:
            lnt = const.tile([P, D], FP32, tag=k, name=f"lnt_{k}")
            src = ln_d[k]
            bcast = bass.AP(tensor=src.tensor, offset=src.offset,
                            ap=[[0, P]] + list(src.ap))
            nc.sync.dma_start(lnt[:], bcast)
            ln_sb[k] = lnt

        def layer_norm4(pool, x_tiles, tag, which):
            """LN of four [P, D] fp32 tiles -> four [P, D] bf16 tiles.
            One ACT Sqrt instruction total (avoids ACT table thrashing)."""
            vb = pool.tile([P, 4], FP32, bufs=1, tag=tag + "vb",
                           name=tag + "vb")
            mvs = []
            for i, x_tile in enumerate(x_tiles):
                xr = x_tile.rearrange("p (g f) -> p g f", g=2)
                stats = pool.tile([P, 2, 6], FP32, bufs=2, tag=tag + "st",
                                  name=tag + "st")
                for g in range(2):
                    nc.vector.bn_stats(out=stats[:, g, :], in_=xr[:, g, :])
                mv = pool.tile([P, 2], FP32, bufs=4, tag=tag + "mv",
                               name=tag + "mv")
                nc.vector.bn_aggr(out=mv[:], in_=stats[:])
                nc.vector.tensor_copy(vb[:, i:i + 1], mv[:, 1:2])
                mvs.append(mv)
            sd = pool.tile([P, 4], FP32, bufs=1, tag=tag + "sd",
                           name=tag + "sd")
            nc.scalar.activation(out=sd[:], in_=vb[:], func=AF.Sqrt,
                                 bias=eps_t[:, 0:1])
            rstd = pool.tile([P, 4], FP32, bufs=1, tag=tag + "rs",
                             name=tag + "rs")
            nc.vector.reciprocal(out=rstd[:], in_=sd[:])
            ys = []
            for i, x_tile in enumerate(x_tiles):
                if apply_ln:
                    yf = pool.tile([P, D], FP32, bufs=2, tag=tag + "yf",
                                   name=tag + "yf")
                    nc.vector.tensor_scalar(out=yf[:], in0=x_tile[:],
                                            scalar1=mvs[i][:, 0:1],
                                            scalar2=rstd[:, i:i + 1],
                                            op0=SUB, op1=MULT)
                    nc.vector.tensor_mul(yf[:], yf[:],
                                         ln_sb[which + "_w"][:])
                    nc.vector.tensor_add(yf[:], yf[:],
                                         ln_sb[which + "_b"][:])
                    y = pool.tile([P, D], BF16, bufs=4, tag=tag + "y",
                                  name=tag + "y")
                    nc.vector.tensor_copy(y[:], yf[:])
                else:
                    y = pool.tile([P, D], BF16, bufs=4, tag=tag + "y",
                                  name=tag + "y")
                    nc.vector.tensor_scalar(out=y[:], in0=x_tile[:],
                                            scalar1=mvs[i][:, 0:1],
                                            scalar2=rstd[:, i:i + 1],
                                            op0=SUB, op1=MULT)
                ys.append(y)
            return ys

        xt = []
        x1 = []
        for st in range(4):
            xti = resid.tile([P, D], FP32, tag=f"xt{st}", name=f"xt{st}")
            xt.append(xti)
            x1t = resid.tile([P, D], FP32, tag=f"x1{st}", name=f"x1_{st}")
            x1.append(x1t)

        # ---------------- P1: LN1 + transpose (dc-major) ----------------
        with tc.tile_pool(name="p1", bufs=1) as p1, \
             tc.tile_pool(name="ps1", bufs=1, space="PSUM") as ps1:
            for st in range(4):
                nc.sync.dma_start(xt[st][:], x_c[st * P:(st + 1) * P, :])
            ys = layer_norm4(p1, xt, "l1", "ln1")
            yT = p1.tile([P, 8, 512], BF16, tag="yT")
            ytc = [dram.tile([4 * P, TOK], BF16, tag=f"ytc{h}",
                             name=f"ytc{h}") for h in range(2)]
            for half in range(2):
                for dc in range(4 * half, 4 * half + 4):
                    for st in range(4):
                        ptt = ps1.tile([P, P], BF16, bufs=2, tag="pt",
                                       name="ptt")
                        nc.tensor.transpose(ptt[:],
                                            ys[st][:, dc * P:(dc + 1) * P],
                                            ident[:])
                        nc.vector.tensor_copy(
                            yT[:, dc, st * P:(st + 1) * P], ptt[:])
                nc.sync.dma_start(
                    ytc[half].rearrange("(dc p) t -> p dc t", p=P),
                    yT[:, 4 * half:4 * half + 4, :])

        # ---------------- P2: AllGather yT (two halves) ----------------
        ytg = []
        for half in range(2):
            g = dram.tile([N_CORES * 4 * P, TOK], BF16, tag=f"ytg{half}",
                          name=f"ytg{half}")
            nc.gpsimd.collective_compute(
                "AllGather", mybir.AluOpType.bypass, replica_groups=RG,
                ins=[ytc[half].opt()], outs=[g.opt()])
            ytg.append(g)

        atc = dram.tile([N_CORES * P, TOK], BF16, tag="atc")

        with tc.tile_pool(name="pwo", bufs=1) as pwo_pool:
            wo_sb = pwo_pool.tile([P, 8, D], BF16, tag="wo")
            nc.sync.dma_start(wo_sb[:],
                              wo_d.rearrange("(dc p) n -> p dc n", p=P))

            with tc.tile_pool(name="p3", bufs=1) as p3:
                masks_sb = p3.tile([P, 4, 512], BF16, tag="masks")
                nc.sync.dma_start(masks_sb[:],
                                  masks_d.rearrange("m p s -> p m s"))
                qTs, kTs = [], []
                for rb in range(8):
                    qt_i = p3.tile([P, 512], BF16, tag=f"qT{rb}",
                                   name=f"qT{rb}")
                    kt_i = p3.tile([P, 512], BF16, tag=f"kT{rb}",
                                   name=f"kT{rb}")
                    qTs.append(qt_i)
                    kTs.append(kt_i)
                vext = p3.tile([P, 32, 130], BF16, tag="vext")

                # ------------ P3: QKV over full sequence ------------
                with tc.tile_pool(name="ps3", bufs=1, space="PSUM") as ps3:
                    for rb in range(8):
                        yts = []
                        for dc in range(8):
                            yt_t = p3.tile([P, 512], BF16, bufs=10,
                                           tag="ytg_t", name="yt_t")
                            half, dd = dc // 4, dc % 4
                            nc.sync.dma_start(
                                yt_t[:],
                                ytg[half][rb * 512 + dd * P:
                                          rb * 512 + (dd + 1) * P, :])
                            yts.append(yt_t)
                        for w_sb, dst in ((wq_sb, qTs[rb]), (wk_sb, kTs[rb])):
                            pq = ps3.tile([P, 512], FP32, bufs=3, tag="pq",
                                          name="pq")
                            for dc in range(8):
                                nc.tensor.matmul(pq[:], w_sb[:, dc, :],
                                                 yts[dc][:],
                                                 start=(dc == 0),
                                                 stop=(dc == 7))
                            nc.scalar.copy(dst[:], pq[:])
                        pv = ps3.tile([P, 512], FP32, bufs=3, tag="pq",
                                      name="pv")
                        for dc in range(8):
                            nc.tensor.matmul(pv[:], wv_sb[:, dc, :],
                                             yts[dc][:], start=(dc == 0),
                                             stop=(dc == 7))
                        vt_tmp = p3.tile([P, 512], BF16, bufs=2, tag="vtt",
                                         name="vt_tmp")
                        nc.scalar.copy(vt_tmp[:], pv[:])
                        for t4 in range(4):
                            tch = rb * 4 + t4
                            pvt = ps3.tile([P, P], BF16, bufs=2, tag="pvt",
                                           name="pvt")
                            nc.tensor.transpose(
                                pvt[:], vt_tmp[:, t4 * P:(t4 + 1) * P],
                                ident[:])
                            nc.vector.tensor_copy(vext[:, tch, 0:64],
                                                  pvt[:, 0:64])
                            nc.vector.tensor_copy(vext[:, tch, 65:129],
                                                  pvt[:, 64:128])
                            nc.vector.tensor_copy(vext[:, tch, 64:65],
                                                  ones_b[:, 0:1])
                            nc.vector.tensor_copy(vext[:, tch, 129:130],
                                                  ones_b[:, 0:1])

                # ------------ P4: attention ------------
                with tc.tile_pool(name="ps4", bufs=1, space="PSUM") as ps4:
                    for b in range(2):
                        for qb in range(4):
                            j_blk = 4 * b + qb
                            qt_blk = qTs[j_blk]
                            nt = 4 * (qb + 1)
                            pa0 = ps4.tile([65, 512], FP32, bufs=1,
                                           tag="pa0", name="pa0")
                            pa1 = ps4.tile([65, 512], FP32, bufs=1,
                                           tag="pa1", name="pa1")
                            pending = []
                            for j in range(nt):
                                tch = 16 * b + j
                                kt_blk = kTs[tch // 4]
                                kc = slice((tch % 4) * P,
                                           (tch % 4 + 1) * P)
                                psc0 = ps4.tile([P, 512], FP32, bufs=3,
                                                tag="sc0", name="psc0")
                                psc1 = ps4.tile([P, 512], FP32, bufs=3,
                                                tag="sc1", name="psc1")
                                nc.tensor.matmul(psc0[:], kt_blk[0:64, kc],
                                                 qt_blk[0:64, :],
                                                 start=True, stop=True)
                                nc.tensor.matmul(psc1[:], kt_blk[64:128, kc],
                                                 qt_blk[64:128, :],
                                                 start=True, stop=True)
                                pt0 = p3.tile([P, 512], BF16, bufs=4,
                                              tag="pt0", name="pt0")
                                pt1 = p3.tile([P, 512], BF16, bufs=4,
                                              tag="pt1", name="pt1")
                                nc.scalar.activation(out=pt0[:], in_=psc0[:],
                                                     func=AF.Exp,
                                                     scale=0.125)
                                nc.scalar.activation(out=pt1[:], in_=psc1[:],
                                                     func=AF.Exp,
                                                     scale=0.125)
                                if j >= nt - 4:
                                    m = j - (nt - 4)
                                    nc.vector.tensor_mul(pt0[:], pt0[:],
                                                         masks_sb[:, m, :])
                                    nc.vector.tensor_mul(pt1[:], pt1[:],
                                                         masks_sb[:, m, :])
                                pending.append((tch, pt0, pt1, j == 0,
                                                j == nt - 1))
                                if len(pending) == 2:
                                    _emit_attn(nc, vext, pa0, pa1,
                                               pending.pop(0))
                            while pending:
                                _emit_attn(nc, vext, pa0, pa1,
                                           pending.pop(0))

                            for hl, pa in ((0, pa0), (1, pa1)):
                                sa = p3.tile([65, 512], FP32, bufs=2,
                                             tag="sa", name="sa")
                                nc.vector.tensor_copy(sa[:], pa[:])
                                rsf = p3.tile([65, 512], FP32, bufs=2,
                                              tag="rsf", name="rsf")
                                nc.vector.reciprocal(out=rsf[64:65, :],
                                                     in_=sa[64:65, :])
                                rsr = p3.tile([65, 512], BF16, bufs=2,
                                              tag="rsr", name="rsr")
                                nc.vector.tensor_copy(rsr[64:65, :],
                                                      rsf[64:65, :])
                                pb = ps4.tile([64, 512], FP32, bufs=3,
                                              tag="sc0", name="pb")
                                nc.tensor.matmul(pb[:], ones_b[64:65, 0:64],
                                                 rsr[64:65, :],
                                                 start=True, stop=True)
                                an = p3.tile([64, 512], BF16, bufs=2,
                                             tag="an", name="an")
                                nc.vector.tensor_mul(an[:], sa[0:64, :],
                                                     pb[:])
                                row = j_blk * P + hl * 64
                                nc.sync.dma_start(atc[row:row + 64, :],
                                                  an[:])

            # ---------------- P5: AllToAll attnT ----------------
            atg = dram.tile([N_CORES * P, TOK], BF16, tag="atg")
            nc.gpsimd.collective_compute(
                "AllToAll", mybir.AluOpType.bypass, replica_groups=RG,
                ins=[atc.opt()], outs=[atg.opt()])

            # ---------------- P6: wo + residual ----------------
            with tc.tile_pool(name="p6", bufs=1) as p6, \
                 tc.tile_pool(name="ps6", bufs=1, space="PSUM") as ps6:
                at2s = []
                for rr in range(8):
                    a2t = p6.tile([P, 512], BF16, tag=f"at2_{rr}",
                                  name=f"at2_{rr}")
                    nc.sync.dma_start(a2t[:],
                                      atg[rr * P:(rr + 1) * P, :])
                    at2s.append(a2t)
                for st in range(4):
                    for ncol in range(2):
                        pw = ps6.tile([P, 512], FP32, bufs=2, tag="pwo",
                                      name="pw")
                        for rr in range(8):
                            nc.tensor.matmul(
                                pw[:], at2s[rr][:, st * P:(st + 1) * P],
                                wo_sb[:, rr, ncol * 512:(ncol + 1) * 512],
                                start=(rr == 0), stop=(rr == 7))
                        nc.vector.tensor_add(
                            x1[st][:, ncol * 512:(ncol + 1) * 512], pw[:],
                            xt[st][:, ncol * 512:(ncol + 1) * 512])

        # ---------------- P7: LN2 + transpose ----------------
        with tc.tile_pool(name="p7", bufs=1) as p7:
            with tc.tile_pool(name="ps7", bufs=1, space="PSUM") as ps7:
                y2T = p7.tile([P, 8, 512], BF16, tag="y2T")
                y2s = layer_norm4(p7, x1, "l2", "ln2")
                for st in range(4):
                    for dc in range(8):
                        ptt2 = ps7.tile([P, P], BF16, bufs=2, tag="pt2",
                                        name="ptt2")
                        nc.tensor.transpose(ptt2[:],
                                            y2s[st][:, dc * P:(dc + 1) * P],
                                            ident[:])
                        nc.vector.tensor_copy(
                            y2T[:, dc, st * P:(st + 1) * P], ptt2[:])

            # ---------------- P8/P9: FFN ----------------
            with tc.tile_pool(name="p8", bufs=1) as p8, \
                 tc.tile_pool(name="ps8", bufs=1, space="PSUM") as ps8:
                hT = p8.tile([P, 32, 512], BF16, tag="hT")
                for fc in range(32):
                    w1tt = p8.tile([P, D], BF16, bufs=4, tag="w1tt",
                                   name="w1tt")
                    nc.sync.dma_start(w1tt[:], w1t_d[fc, :, :])
                    ph = ps8.tile([P, 512], FP32, bufs=2, tag="ph",
                                  name="ph")
                    for dc in range(8):
                        nc.tensor.matmul(ph[:],
                                         w1tt[:, dc * P:(dc + 1) * P],
                                         y2T[:, dc, :], start=(dc == 0),
                                         stop=(dc == 7))
                    nc.scalar.activation(out=hT[:, fc, :], in_=ph[:],
                                         func=AF.Gelu_apprx_tanh,
                                         bias=b1_sb[:, fc:fc + 1])
                for ncol in range(2):
                    nc2 = slice(ncol * 512, (ncol + 1) * 512)
                    po = [ps8.tile([P, 512], FP32, bufs=1, tag=f"po{sc}",
                                   name=f"po_{ncol}_{sc}")
                          for sc in range(4)]
                    for sc in range(4):
                        nc.tensor.matmul(po[sc][:], ones_b[0:1, :],
                                         b2b[0:1, nc2], start=True,
                                         stop=False)
                    for fc in range(32):
                        w2tt = p8.tile([P, 512], BF16, bufs=4, tag="w2tt",
                                       name="w2tt")
                        nc.sync.dma_start(w2tt[:],
                                          w2t_d[fc * P:(fc + 1) * P, nc2])
                        for sc in range(4):
                            nc.tensor.matmul(
                                po[sc][:], hT[:, fc, sc * P:(sc + 1) * P],
                                w2tt[:], start=False, stop=(fc == 31))
                    for sc in range(4):
                        oh = p8.tile([P, 512], FP32, bufs=2, tag="oh",
                                     name="oh")
                        nc.vector.tensor_add(oh[:], po[sc][:],
                                             x1[sc][:, nc2])
                        nc.sync.dma_start(out_d[sc * P:(sc + 1) * P, nc2],
                                          oh[:])

    nc.compile()
    return nc


def _emit_attn(nc, vext, pa0, pa1, item):
    tch, pt0, pt1, is_first, is_last = item
    nc.tensor.matmul(pa0[:], vext[:, tch, 0:65], pt0[:],
                     start=is_first, stop=is_last)
    nc.tensor.matmul(pa1[:], vext[:, tch, 65:130], pt1[:],
                     start=is_first, stop=is_last)


def _get_nc(apply_ln):
    key = ("nc_v3", apply_ln)
    if key not in _CACHE:
        _CACHE[key] = _build(apply_ln)
    return _CACHE[key]


def _make_masks():
    tt = np.arange(P)[:, None]
    ss = np.arange(512)[None, :]
    return np.stack([(P * m + tt <= ss) for m in range(4)]
                    ).astype(np.float32)


def _bf16(a):
    import ml_dtypes
    return np.asarray(a, dtype=np.float32).astype(ml_dtypes.bfloat16)


def _prepare(inputs):
    x = np.asarray(inputs["x"], dtype=np.float32).reshape(T, D)
    wq = np.asarray(inputs["wq"], dtype=np.float32)
    wk = np.asarray(inputs["wk"], dtype=np.float32)
    wv = np.asarray(inputs["wv"], dtype=np.float32)
    wo = _bf16(inputs["wo"])
    w1 = np.asarray(inputs["w1"], dtype=np.float32)            # [DFF, D]
    # w1tile[fc, p, dc*128+m] = w1[fc*128+m, dc*128+p]
    w1tile = _bf16(np.ascontiguousarray(
        w1.reshape(32, P, 8, P).transpose(0, 3, 2, 1)
        .reshape(32, P, D)))
    w2t = _bf16(np.asarray(inputs["w2"], dtype=np.float32).T)   # [DFF, D]
    b1 = np.asarray(inputs["b1"], dtype=np.float32)
    b2 = np.asarray(inputs["b2"], dtype=np.float32)
    masks = _bf16(_make_masks())

    apply_ln = not (
        np.all(np.asarray(inputs["ln1_w"]) == 1)
        and np.all(np.asarray(inputs["ln1_b"]) == 0)
        and np.all(np.asarray(inputs["ln2_w"]) == 1)
        and np.all(np.asarray(inputs["ln2_b"]) == 0))

    in_maps = []
    for r in range(N_CORES):
        m = {
            "x_c": np.ascontiguousarray(x[r * TOK:(r + 1) * TOK]),
            "wq_c": _bf16(np.concatenate([wq[2 * r], wq[2 * r + 1]],
                                         axis=1)),
            "wk_c": _bf16(np.concatenate([wk[2 * r], wk[2 * r + 1]],
                                         axis=1)),
            "wv_c": _bf16(np.concatenate([wv[2 * r], wv[2 * r + 1]],
                                         axis=1)),
            "wo": wo, "w1tile": w1tile, "w2t": w2t, "b1": b1, "b2": b2,
            "masks": masks,
        }
        if apply_ln:
            for k in ("ln1_w", "ln1_b", "ln2_w", "ln2_b"):
                m[k] = np.asarray(inputs[k], dtype=np.float32)
        in_maps.append(m)
    return in_maps, apply_ln


def _run(inputs, trace=False):
    from concourse.bass_utils import run_bass_kernel_spmd
    in_maps, apply_ln = _prepare(inputs)
    nc = _get_nc(apply_ln)
    res = run_bass_kernel_spmd(nc, in_maps, list(range(N_CORES)),
                               trace=trace)
    out = np.concatenate([res.results[r]["out"] for r in range(N_CORES)],
                         axis=0).reshape(B, S, D).astype(np.float32)
    return out, res


def kernel(**inputs):
    out, _ = _run(inputs)
    return out


def bench(**inputs):
    """Like kernel() but with NTFF tracing; returns (out, exec_time_ns)."""
    out, res = _run(inputs, trace=True)
    return out, res.exec_time_ns


# revision 13
# speedup vs baseline: 1.3146x; 1.0305x over previous
"""Trainium2 Bass kernel for a dense transformer decoder layer.

Strategy (8 NeuronCores, SPMD, uniform program):
  - Tokens (flattened batch*seq = 4096) are sharded 512/core for LayerNorm,
    wo-projection, and the FFN.
  - Attention is sharded over heads: core r owns heads {2r, 2r+1} over the
    full sequence (uniform causal block structure on every core).
  - Collective 1: AllGather (split in two halves, overlapped with QKV) of
    the LN1 output, transposed (feature-major).
  - Collective 2: AllToAll converting head-sharded attention output into
    token-sharded full-head attnT (feeds the wo matmul directly as lhsT).
  - Matmul operands are bf16; accumulation, LayerNorm, softmax and
    residual math stays fp32 in PSUM/SBUF.
"""
import numpy as np

N_CORES = 8
B, S, D, H, E, DFF = 2, 2048, 1024, 16, 64, 4096
T = B * S              # 4096 flat tokens
TOK = T // N_CORES     # 512 tokens per core
P = 128
EPS = 1e-5

_CACHE = {}


def _build(apply_ln):
    from contextlib import ExitStack
    import concourse.bass as bass
    import concourse.tile as tile
    from concourse import bacc, mybir
    from concourse.masks import make_identity

    FP32 = mybir.dt.float32
    BF16 = mybir.dt.bfloat16
    AF = mybir.ActivationFunctionType
    SUB = mybir.AluOpType.subtract
    MULT = mybir.AluOpType.mult

    nc = bacc.Bacc("TRN2", target_bir_lowering=False, debug=False,
                   num_devices=N_CORES)

    x_c = nc.dram_tensor("x_c", [TOK, D], FP32, kind="ExternalInput").ap()
    wq_c = nc.dram_tensor("wq_c", [D, P], BF16, kind="ExternalInput").ap()
    wk_c = nc.dram_tensor("wk_c", [D, P], BF16, kind="ExternalInput").ap()
    wv_c = nc.dram_tensor("wv_c", [D, P], BF16, kind="ExternalInput").ap()
    wo_d = nc.dram_tensor("wo", [D, D], BF16, kind="ExternalInput").ap()
    # w1tile[fc, p, dc*128+m] = w1[fc*128+m, dc*128+p]
    w1t_d = nc.dram_tensor("w1tile", [32, P, D], BF16,
                           kind="ExternalInput").ap()
    w2t_d = nc.dram_tensor("w2t", [DFF, D], BF16, kind="ExternalInput").ap()
    b1_d = nc.dram_tensor("b1", [DFF], FP32, kind="ExternalInput").ap()
    b2_d = nc.dram_tensor("b2", [D], FP32, kind="ExternalInput").ap()
    masks_d = nc.dram_tensor("masks", [4, P, 512], BF16,
                             kind="ExternalInput").ap()
    ln_d = {}
    if apply_ln:
        ln_d = {k: nc.dram_tensor(k, [D], FP32, kind="ExternalInput").ap()
                for k in ("ln1_w", "ln1_b", "ln2_w", "ln2_b")}
    out_d = nc.dram_tensor("out", [TOK, D], FP32, kind="ExternalOutput").ap()

    RG = [list(range(N_CORES))]

    with tile.TileContext(nc) as tc, ExitStack() as top:
        const = top.enter_context(tc.tile_pool(name="const", bufs=1))
        resid = top.enter_context(tc.tile_pool(name="resid", bufs=1))
        dram = top.enter_context(tc.tile_pool(name="dram", bufs=1,
                                              space="DRAM"))

        # ---------------- constants ----------------
        ident_f = const.tile([P, P], FP32, tag="ident_f")
        make_identity(nc, ident_f)
        ident = const.tile([P, P], BF16, tag="ident")
        nc.vector.tensor_copy(ident[:], ident_f[:])
        ones_f = const.tile([P, 128], FP32, tag="ones_f")
        nc.vector.memset(ones_f[:], 1.0)
        ones_b = const.tile([P, 128], BF16, tag="ones_b")
        nc.vector.tensor_copy(ones_b[:], ones_f[:])
        eps_t = const.tile([P, 1], FP32, tag="eps")
        nc.vector.memset(eps_t[:], EPS)

        wq_sb = const.tile([P, 8, P], BF16, tag="wq")
        wk_sb = const.tile([P, 8, P], BF16, tag="wk")
        wv_sb = const.tile([P, 8, P], BF16, tag="wv")
        for w_sb, w_ap in ((wq_sb, wq_c), (wk_sb, wk_c), (wv_sb, wv_c)):
            nc.sync.dma_start(
                w_sb[:], w_ap.rearrange("(dc p) m -> p dc m", p=P))

        # b1 laid out [P, 32]: element (p, fc) = b1[fc*128 + p]  (ACT bias)
        b1_sb = const.tile([P, 32], FP32, tag="b1")
        nc.sync.dma_start(b1_sb[:], b1_d.rearrange("(fc p) -> p fc", p=P))
        b2f = const.tile([1, D], FP32, tag="b2f")
        nc.sync.dma_start(b2f[:], b2_d.rearrange("(o f) -> o f", o=1))
        b2b = const.tile([1, D], BF16, tag="b2b")
        nc.vector.tensor_copy(b2b[:], b2f[:])

        ln_sb = {}
        for k in ln_d:
            lnt = const.tile([P, D], FP32, tag=k, name=f"lnt_{k}")
            src = ln_d[k]
            bcast = bass.AP(tensor=src.tensor, offset=src.offset,
                            ap=[[0, P]] + list(src.ap))
            nc.sync.dma_start(lnt[:], bcast)
            ln_sb[k] = lnt

        def layer_norm4(pool, x_tiles, tag, which):
            """LN of four [P, D] fp32 tiles -> four [P, D] bf16 tiles.
            One ACT Sqrt instruction total (avoids ACT table thrashing)."""
            vb = pool.tile([P, 4], FP32, bufs=1, tag=tag + "vb",
                           name=tag + "vb")
            mvs = []
            for i, x_tile in enumerate(x_tiles):
                xr = x_tile.rearrange("p (g f) -> p g f", g=2)
                stats = pool.tile([P, 2, 6], FP32, bufs=2, tag=tag + "st",
                                  name=tag + "st")
                for g in range(2):
                    nc.vector.bn_stats(out=stats[:, g, :], in_=xr[:, g, :])
                mv = pool.tile([P, 2], FP32, bufs=4, tag=tag + "mv",
                               name=tag + "mv")
                nc.vector.bn_aggr(out=mv[:], in_=stats[:])
                nc.vector.tensor_copy(vb[:, i:i + 1], mv[:, 1:2])
                mvs.append(mv)
            sd = pool.tile([P, 4], FP32, bufs=1, tag=tag + "sd",
                           name=tag + "sd")
            nc.scalar.activation(out=sd[:], in_=vb[:], func=AF.Sqrt,
                                 bias=eps_t[:, 0:1])
            rstd = pool.tile([P, 4], FP32, bufs=1, tag=tag + "rs",
                             name=tag + "rs")
            nc.vector.reciprocal(out=rstd[:], in_=sd[:])
            ys = []
            for i, x_tile in enumerate(x_tiles):
                if apply_ln:
                    yf = pool.tile([P, D], FP32, bufs=2, tag=tag + "yf",
                                   name=tag + "yf")
                    nc.vector.tensor_scalar(out=yf[:], in0=x_tile[:],
                                            scalar1=mvs[i][:, 0:1],
                                            scalar2=rstd[:, i:i + 1],
                                            op0=SUB, op1=MULT)
                    nc.vector.tensor_mul(yf[:], yf[:],
                                         ln_sb[which + "_w"][:])
                    nc.vector.tensor_add(yf[:], yf[:],
                                         ln_sb[which + "_b"][:])
                    y = pool.tile([P, D], BF16, bufs=4, tag=tag + "y",
                                  name=tag + "y")
                    nc.vector.tensor_copy(y[:], yf[:])
                else:
                    y = pool.tile([P, D], BF16, bufs=4, tag=tag + "y",
                                  name=tag + "y")
                    nc.vector.tensor_scalar(out=y[:], in0=x_tile[:],
                                            scalar1=mvs[i][:, 0:1],
                                            scalar2=rstd[:, i:i + 1],
                                            op0=SUB, op1=MULT)
                ys.append(y)
            return ys

        xt = []
        x1 = []
        for st in range(4):
            xti = resid.tile([P, D], FP32, tag=f"xt{st}", name=f"xt{st}")
            xt.append(xti)
            x1t = resid.tile([P, D], FP32, tag=f"x1{st}", name=f"x1_{st}")
            x1.append(x1t)

        # ---------------- P1: LN1 + transpose (dc-major) ----------------
        with tc.tile_pool(name="p1", bufs=1) as p1, \
             tc.tile_pool(name="ps1", bufs=1, space="PSUM") as ps1:
            for st in range(4):
                nc.sync.dma_start(xt[st][:], x_c[st * P:(st + 1) * P, :])
            ys = layer_norm4(p1, xt, "l1", "ln1")
            yT = p1.tile([P, 8, 512], BF16, tag="yT")
            ytc = dram.tile([D, TOK], BF16, tag="ytc")
            for dc in range(8):
                for st in range(4):
                    ptt = ps1.tile([P, P], BF16, bufs=2, tag="pt",
                                   name="ptt")
                    nc.tensor.transpose(ptt[:],
                                        ys[st][:, dc * P:(dc + 1) * P],
                                        ident[:])
                    nc.vector.tensor_copy(
                        yT[:, dc, st * P:(st + 1) * P], ptt[:])
            nc.sync.dma_start(ytc.rearrange("(dc p) t -> p dc t", p=P),
                              yT[:])

        # ---------------- P2: AllGather yT ----------------
        ytg = dram.tile([N_CORES * D, TOK], BF16, tag="ytg")
        nc.gpsimd.collective_compute(
            "AllGather", mybir.AluOpType.bypass, replica_groups=RG,
            ins=[ytc.opt()], outs=[ytg.opt()])

        atc = dram.tile([N_CORES * P, TOK], BF16, tag="atc")

        with tc.tile_pool(name="pwo", bufs=1) as pwo_pool:
            wo_sb = pwo_pool.tile([P, 8, D], BF16, tag="wo")
            nc.sync.dma_start(wo_sb[:],
                              wo_d.rearrange("(dc p) n -> p dc n", p=P))

            with tc.tile_pool(name="p3", bufs=1) as p3:
                masks_sb = p3.tile([P, 4, 512], BF16, tag="masks")
                nc.sync.dma_start(masks_sb[:],
                                  masks_d.rearrange("m p s -> p m s"))
                qTs, kTs = [], []
                for rb in range(8):
                    qt_i = p3.tile([P, 512], BF16, tag=f"qT{rb}",
                                   name=f"qT{rb}")
                    kt_i = p3.tile([P, 512], BF16, tag=f"kT{rb}",
                                   name=f"kT{rb}")
                    qTs.append(qt_i)
                    kTs.append(kt_i)
                vext = p3.tile([P, 32, 130], BF16, tag="vext")

                # ------------ P3: QKV over full sequence ------------
                with tc.tile_pool(name="ps3", bufs=1, space="PSUM") as ps3:
                    for rb in range(8):
                        yts = []
                        for dc in range(8):
                            yt_t = p3.tile([P, 512], BF16, bufs=10,
                                           tag="ytg_t", name="yt_t")
                            base = rb * D + dc * P
                            nc.sync.dma_start(yt_t[:],
                                              ytg[base:base + P, :])
                            yts.append(yt_t)
                        for w_sb, dst in ((wq_sb, qTs[rb]), (wk_sb, kTs[rb])):
                            pq = ps3.tile([P, 512], FP32, bufs=3, tag="pq",
                                          name="pq")
                            for dc in range(8):
                                nc.tensor.matmul(pq[:], w_sb[:, dc, :],
                                                 yts[dc][:],
                                                 start=(dc == 0),
                                                 stop=(dc == 7))
                            nc.scalar.copy(dst[:], pq[:])
                        pv = ps3.tile([P, 512], FP32, bufs=3, tag="pq",
                                      name="pv")
                        for dc in range(8):
                            nc.tensor.matmul(pv[:], wv_sb[:, dc, :],
                                             yts[dc][:], start=(dc == 0),
                                             stop=(dc == 7))
                        vt_tmp = p3.tile([P, 512], BF16, bufs=2, tag="vtt",
                                         name="vt_tmp")
                        nc.scalar.copy(vt_tmp[:], pv[:])
                        for t4 in range(4):
                            tch = rb * 4 + t4
                            pvt = ps3.tile([P, P], BF16, bufs=2, tag="pvt",
                                           name="pvt")
                            nc.tensor.transpose(
                                pvt[:], vt_tmp[:, t4 * P:(t4 + 1) * P],
                                ident[:])
                            nc.vector.tensor_copy(vext[:, tch, 0:64],
                                                  pvt[:, 0:64])
                            nc.vector.tensor_copy(vext[:, tch, 65:129],
                                                  pvt[:, 64:128])
                            nc.vector.tensor_copy(vext[:, tch, 64:65],
                                                  ones_b[:, 0:1])
                            nc.vector.tensor_copy(vext[:, tch, 129:130],
                                                  ones_b[:, 0:1])

                # ------------ P4: attention ------------
                with tc.tile_pool(name="ps4", bufs=1, space="PSUM") as ps4:
                    for b in range(2):
                        for qb in range(4):
                            j_blk = 4 * b + qb
                            qt_blk = qTs[j_blk]
                            nt = 4 * (qb + 1)
                            pts = []
                            for j in range(nt):
                                tch = 16 * b + j
                                kt_blk = kTs[tch // 4]
                                kc = slice((tch % 4) * P,
                                           (tch % 4 + 1) * P)
                                psc0 = ps4.tile([P, 512], FP32, bufs=3,
                                                tag="sc0", name="psc0")
                                psc1 = ps4.tile([P, 512], FP32, bufs=3,
                                                tag="sc1", name="psc1")
                                nc.tensor.matmul(psc0[:], kt_blk[0:64, kc],
                                                 qt_blk[0:64, :],
                                                 start=True, stop=True)
                                nc.tensor.matmul(psc1[:], kt_blk[64:128, kc],
                                                 qt_blk[64:128, :],
                                                 start=True, stop=True)
                                pt0 = p3.tile([P, 512], BF16, bufs=20,
                                              tag="pt0", name="pt0")
                                pt1 = p3.tile([P, 512], BF16, bufs=20,
                                              tag="pt1", name="pt1")
                                nc.scalar.activation(out=pt0[:], in_=psc0[:],
                                                     func=AF.Exp,
                                                     scale=0.125)
                                nc.scalar.activation(out=pt1[:], in_=psc1[:],
                                                     func=AF.Exp,
                                                     scale=0.125)
                                if j >= nt - 4:
                                    m = j - (nt - 4)
                                    nc.vector.tensor_mul(pt0[:], pt0[:],
                                                         masks_sb[:, m, :])
                                    nc.vector.tensor_mul(pt1[:], pt1[:],
                                                         masks_sb[:, m, :])
                                pts.append((tch, pt0, pt1))
                            pa0 = ps4.tile([65, 512], FP32, bufs=1,
                                           tag="pa0", name="pa0")
                            pa1 = ps4.tile([65, 512], FP32, bufs=1,
                                           tag="pa1", name="pa1")
                            for idx, (tch, pt0, pt1) in enumerate(pts):
                                nc.tensor.matmul(pa0[:],
                                                 vext[:, tch, 0:65],
                                                 pt0[:], start=(idx == 0),
                                                 stop=(idx == nt - 1))
                                nc.tensor.matmul(pa1[:],
                                                 vext[:, tch, 65:130],
                                                 pt1[:], start=(idx == 0),
                                                 stop=(idx == nt - 1))

                            for hl, pa in ((0, pa0), (1, pa1)):
                                sa = p3.tile([65, 512], FP32, bufs=2,
                                             tag="sa", name="sa")
                                nc.vector.tensor_copy(sa[:], pa[:])
                                rsf = p3.tile([65, 512], FP32, bufs=2,
                                              tag="rsf", name="rsf")
                                nc.vector.reciprocal(out=rsf[64:65, :],
                                                     in_=sa[64:65, :])
                                rsr = p3.tile([65, 512], BF16, bufs=2,
                                              tag="rsr", name="rsr")
                                nc.vector.tensor_copy(rsr[64:65, :],
                                                      rsf[64:65, :])
                                pb = ps4.tile([64, 512], FP32, bufs=3,
                                              tag="sc0", name="pb")
                                nc.tensor.matmul(pb[:], ones_b[64:65, 0:64],
                                                 rsr[64:65, :],
                                                 start=True, stop=True)
                                an = p3.tile([64, 512], BF16, bufs=2,
                                             tag="an", name="an")
                                nc.vector.tensor_mul(an[:], sa[0:64, :],
                                                     pb[:])
                                row = j_blk * P + hl * 64
                                nc.sync.dma_start(atc[row:row + 64, :],
                                                  an[:])

            # ---------------- P5: AllToAll attnT ----------------
            atg = dram.tile([N_CORES * P, TOK], BF16, tag="atg")
            nc.gpsimd.collective_compute(
                "AllToAll", mybir.AluOpType.bypass, replica_groups=RG,
                ins=[atc.opt()], outs=[atg.opt()])

            # ---------------- P6: wo + residual ----------------
            with tc.tile_pool(name="p6", bufs=1) as p6, \
                 tc.tile_pool(name="ps6", bufs=1, space="PSUM") as ps6:
                at2s = []
                for rr in range(8):
                    a2t = p6.tile([P, 512], BF16, tag=f"at2_{rr}",
                                  name=f"at2_{rr}")
                    nc.sync.dma_start(a2t[:],
                                      atg[rr * P:(rr + 1) * P, :])
                    at2s.append(a2t)
                for st in range(4):
                    for ncol in range(2):
                        pw = ps6.tile([P, 512], FP32, bufs=2, tag="pwo",
                                      name="pw")
                        for rr in range(8):
                            nc.tensor.matmul(
                                pw[:], at2s[rr][:, st * P:(st + 1) * P],
                                wo_sb[:, rr, ncol * 512:(ncol + 1) * 512],
                                start=(rr == 0), stop=(rr == 7))
                        nc.vector.tensor_add(
                            x1[st][:, ncol * 512:(ncol + 1) * 512], pw[:],
                            xt[st][:, ncol * 512:(ncol + 1) * 512])

        # ---------------- P7: LN2 + transpose ----------------
        with tc.tile_pool(name="p7", bufs=1) as p7:
            with tc.tile_pool(name="ps7", bufs=1, space="PSUM") as ps7:
                y2T = p7.tile([P, 8, 512], BF16, tag="y2T")
                y2s = layer_norm4(p7, x1, "l2", "ln2")
                for st in range(4):
                    for dc in range(8):
                        ptt2 = ps7.tile([P, P], BF16, bufs=2, tag="pt2",
                                        name="ptt2")
                        nc.tensor.transpose(ptt2[:],
                                            y2s[st][:, dc * P:(dc + 1) * P],
                                            ident[:])
                        nc.vector.tensor_copy(
                            y2T[:, dc, st * P:(st + 1) * P], ptt2[:])

            # ---------------- P8/P9: FFN ----------------
            with tc.tile_pool(name="p8", bufs=1) as p8, \
                 tc.tile_pool(name="ps8", bufs=1, space="PSUM") as ps8:
                hT = p8.tile([P, 32, 512], BF16, tag="hT")
                for fc in range(32):
                    w1tt = p8.tile([P, D], BF16, bufs=4, tag="w1tt",
                                   name="w1tt")
                    nc.sync.dma_start(w1tt[:], w1t_d[fc, :, :])
                    ph = ps8.tile([P, 512], FP32, bufs=3, tag="ph",
                                  name="ph")
                    for dc in range(8):
                        nc.tensor.matmul(ph[:],
                                         w1tt[:, dc * P:(dc + 1) * P],
                                         y2T[:, dc, :], start=(dc == 0),
                                         stop=(dc == 7))
                    nc.scalar.activation(out=hT[:, fc, :], in_=ph[:],
                                         func=AF.Gelu_apprx_tanh,
                                         bias=b1_sb[:, fc:fc + 1])
                for ncol in range(2):
                    nc2 = slice(ncol * 512, (ncol + 1) * 512)
                    po = [ps8.tile([P, 512], FP32, bufs=1, tag=f"po{sc}",
                                   name=f"po_{ncol}_{sc}")
                          for sc in range(4)]
                    for sc in range(4):
                        nc.tensor.matmul(po[sc][:], ones_b[0:1, :],
                                         b2b[0:1, nc2], start=True,
                                         stop=False)
                    for fc in range(32):
                        w2tt = p8.tile([P, 512], BF16, bufs=4, tag="w2tt",
                                       name="w2tt")
                        nc.sync.dma_start(w2tt[:],
                                          w2t_d[fc * P:(fc + 1) * P, nc2])
                        for sc in range(4):
                            nc.tensor.matmul(
                                po[sc][:], hT[:, fc, sc * P:(sc + 1) * P],
                                w2tt[:], start=False, stop=(fc == 31))
                    for sc in range(4):
                        oh = p8.tile([P, 512], FP32, bufs=2, tag="oh",
                                     name="oh")
                        nc.vector.tensor_add(oh[:], po[sc][:],
                                             x1[sc][:, nc2])
                        nc.sync.dma_start(out_d[sc * P:(sc + 1) * P, nc2],
                                          oh[:])

    nc.compile()
    return nc


def _emit_attn(nc, vext, pa0, pa1, item):
    tch, pt0, pt1, is_first, is_last = item
    nc.tensor.matmul(pa0[:], vext[:, tch, 0:65], pt0[:],
                     start=is_first, stop=is_last)
    nc.tensor.matmul(pa1[:], vext[:, tch, 65:130], pt1[:],
                     start=is_first, stop=is_last)


def _get_nc(apply_ln):
    key = ("nc_v4", apply_ln)
    if key not in _CACHE:
        _CACHE[key] = _build(apply_ln)
    return _CACHE[key]


def _make_masks():
    tt = np.arange(P)[:, None]
    ss = np.arange(512)[None, :]
    return np.stack([(P * m + tt <= ss) for m in range(4)]
                    ).astype(np.float32)


def _bf16(a):
    import ml_dtypes
    return np.asarray(a, dtype=np.float32).astype(ml_dtypes.bfloat16)


def _prepare(inputs):
    x = np.asarray(inputs["x"], dtype=np.float32).reshape(T, D)
    wq = np.asarray(inputs["wq"], dtype=np.float32)
    wk = np.asarray(inputs["wk"], dtype=np.float32)
    wv = np.asarray(inputs["wv"], dtype=np.float32)
    wo = _bf16(inputs["wo"])
    w1 = np.asarray(inputs["w1"], dtype=np.float32)            # [DFF, D]
    # w1tile[fc, p, dc*128+m] = w1[fc*128+m, dc*128+p]
    w1tile = _bf16(np.ascontiguousarray(
        w1.reshape(32, P, 8, P).transpose(0, 3, 2, 1)
        .reshape(32, P, D)))
    w2t = _bf16(np.asarray(inputs["w2"], dtype=np.float32).T)   # [DFF, D]
    b1 = np.asarray(inputs["b1"], dtype=np.float32)
    b2 = np.asarray(inputs["b2"], dtype=np.float32)
    masks = _bf16(_make_masks())

    apply_ln = not (
        np.all(np.asarray(inputs["ln1_w"]) == 1)
        and np.all(np.asarray(inputs["ln1_b"]) == 0)
        and np.all(np.asarray(inputs["ln2_w"]) == 1)
        and np.all(np.asarray(inputs["ln2_b"]) == 0))

    in_maps = []
    for r in range(N_CORES):
        m = {
            "x_c": np.ascontiguousarray(x[r * TOK:(r + 1) * TOK]),
            "wq_c": _bf16(np.concatenate([wq[2 * r], wq[2 * r + 1]],
                                         axis=1)),
            "wk_c": _bf16(np.concatenate([wk[2 * r], wk[2 * r + 1]],
                                         axis=1)),
            "wv_c": _bf16(np.concatenate([wv[2 * r], wv[2 * r + 1]],
                                         axis=1)),
            "wo": wo, "w1tile": w1tile, "w2t": w2t, "b1": b1, "b2": b2,
            "masks": masks,
        }
        if apply_ln:
            for k in ("ln1_w", "ln1_b", "ln2_w", "ln2_b"):
                m[k] = np.asarray(inputs[k], dtype=np.float32)
        in_maps.append(m)
    return in_maps, apply_ln


def _run(inputs, trace=False):
    from concourse.bass_utils import run_bass_kernel_spmd
    in_maps, apply_ln = _prepare(inputs)
    nc = _get_nc(apply_ln)
    res = run_bass_kernel_spmd(nc, in_maps, list(range(N_CORES)),
                               trace=trace)
    out = np.concatenate([res.results[r]["out"] for r in range(N_CORES)],
                         axis=0).reshape(B, S, D).astype(np.float32)
    return out, res


def kernel(**inputs):
    out, _ = _run(inputs)
    return out


def bench(**inputs):
    """Like kernel() but with NTFF tracing; returns (out, exec_time_ns)."""
    out, res = _run(inputs, trace=True)
    return out, res.exec_time_ns


# revision 15
# speedup vs baseline: 1.3309x; 1.0124x over previous
"""Trainium2 Bass kernel for a dense transformer decoder layer.

Strategy (8 NeuronCores, SPMD, uniform program):
  - Tokens (flattened batch*seq = 4096) are sharded 512/core for LayerNorm,
    wo-projection, and the FFN.
  - Attention is sharded over heads: core r owns heads {2r, 2r+1} over the
    full sequence (uniform causal block structure on every core).
  - Collective 1: AllGather (split in two halves, overlapped with QKV) of
    the LN1 output, transposed (feature-major).
  - Collective 2: AllToAll converting head-sharded attention output into
    token-sharded full-head attnT (feeds the wo matmul directly as lhsT).
  - Matmul operands are bf16; accumulation, LayerNorm, softmax and
    residual math stays fp32 in PSUM/SBUF.
"""
import numpy as np

N_CORES = 8
B, S, D, H, E, DFF = 2, 2048, 1024, 16, 64, 4096
T = B * S              # 4096 flat tokens
TOK = T // N_CORES     # 512 tokens per core
P = 128
EPS = 1e-5

_CACHE = {}


def _build(apply_ln):
    from contextlib import ExitStack
    import concourse.bass as bass
    import concourse.tile as tile
    from concourse import bacc, mybir
    from concourse.masks import make_identity

    FP32 = mybir.dt.float32
    BF16 = mybir.dt.bfloat16
    AF = mybir.ActivationFunctionType
    SUB = mybir.AluOpType.subtract
    MULT = mybir.AluOpType.mult

    nc = bacc.Bacc("TRN2", target_bir_lowering=False, debug=False,
                   num_devices=N_CORES)

    x_c = nc.dram_tensor("x_c", [TOK, D], FP32, kind="ExternalInput").ap()
    wq_c = nc.dram_tensor("wq_c", [D, P], BF16, kind="ExternalInput").ap()
    wk_c = nc.dram_tensor("wk_c", [D, P], BF16, kind="ExternalInput").ap()
    wv_c = nc.dram_tensor("wv_c", [D, P], BF16, kind="ExternalInput").ap()
    wo_d = nc.dram_tensor("wo", [D, D], BF16, kind="ExternalInput").ap()
    # w1tile[fc, p, dc*128+m] = w1[fc*128+m, dc*128+p]
    w1t_d = nc.dram_tensor("w1tile", [32, P, D], BF16,
                           kind="ExternalInput").ap()
    w2t_d = nc.dram_tensor("w2t", [DFF, D], BF16, kind="ExternalInput").ap()
    b1_d = nc.dram_tensor("b1", [DFF], FP32, kind="ExternalInput").ap()
    b2_d = nc.dram_tensor("b2", [D], FP32, kind="ExternalInput").ap()
    masks_d = nc.dram_tensor("masks", [4, P, 512], BF16,
                             kind="ExternalInput").ap()
    ln_d = {}
    if apply_ln:
        ln_d = {k: nc.dram_tensor(k, [D], FP32, kind="ExternalInput").ap()
                for k in ("ln1_w", "ln1_b", "ln2_w", "ln2_b")}
    out_d = nc.dram_tensor("out", [TOK, D], FP32, kind="ExternalOutput").ap()

    RG = [list(range(N_CORES))]

    with tile.TileContext(nc) as tc, ExitStack() as top:
        const = top.enter_context(tc.tile_pool(name="const", bufs=1))
        resid = top.enter_context(tc.tile_pool(name="resid", bufs=1))
        dram = top.enter_context(tc.tile_pool(name="dram", bufs=1,
                                              space="DRAM"))

        # ---------------- constants ----------------
        ident_f = const.tile([P, P], FP32, tag="ident_f")
        make_identity(nc, ident_f)
        ident = const.tile([P, P], BF16, tag="ident")
        nc.vector.tensor_copy(ident[:], ident_f[:])
        ones_f = const.tile([P, 128], FP32, tag="ones_f")
        nc.vector.memset(ones_f[:], 1.0)
        ones_b = const.tile([P, 128], BF16, tag="ones_b")
        nc.vector.tensor_copy(ones_b[:], ones_f[:])
        eps_t = const.tile([P, 1], FP32, tag="eps")
        nc.vector.memset(eps_t[:], EPS)
        zf = const.tile([P, 512], FP32, tag="zf")
        nc.vector.memset(zf[:], 0.0)
        zeros_b = const.tile([P, 512], BF16, tag="zeros_b")
        nc.vector.tensor_copy(zeros_b[:], zf[:])

        wq_sb = const.tile([P, 8, P], BF16, tag="wq")
        wk_sb = const.tile([P, 8, P], BF16, tag="wk")
        wv_sb = const.tile([P, 8, P], BF16, tag="wv")
        for w_sb, w_ap in ((wq_sb, wq_c), (wk_sb, wk_c), (wv_sb, wv_c)):
            nc.sync.dma_start(
                w_sb[:], w_ap.rearrange("(dc p) m -> p dc m", p=P))

        # b1 laid out [P, 32]: element (p, fc) = b1[fc*128 + p]  (ACT bias)
        b1_sb = const.tile([P, 32], FP32, tag="b1")
        nc.sync.dma_start(b1_sb[:], b1_d.rearrange("(fc p) -> p fc", p=P))
        b2f = const.tile([1, D], FP32, tag="b2f")
        nc.sync.dma_start(b2f[:], b2_d.rearrange("(o f) -> o f", o=1))
        b2b = const.tile([1, D], BF16, tag="b2b")
        nc.vector.tensor_copy(b2b[:], b2f[:])

        ln_sb = {}
        for k in ln_d:
            lnt = const.tile([P, D], FP32, tag=k, name=f"lnt_{k}")
            src = ln_d[k]
            bcast = bass.AP(tensor=src.tensor, offset=src.offset,
                            ap=[[0, P]] + list(src.ap))
            nc.sync.dma_start(lnt[:], bcast)
            ln_sb[k] = lnt

        def layer_norm4(pool, x_tiles, tag, which):
            """LN of four [P, D] fp32 tiles -> four [P, D] bf16 tiles.
            One ACT Sqrt instruction total (avoids ACT table thrashing)."""
            vb = pool.tile([P, 4], FP32, bufs=1, tag=tag + "vb",
                           name=tag + "vb")
            mvs = []
            for i, x_tile in enumerate(x_tiles):
                xr = x_tile.rearrange("p (g f) -> p g f", g=2)
                stats = pool.tile([P, 2, 6], FP32, bufs=2, tag=tag + "st",
                                  name=tag + "st")
                for g in range(2):
                    nc.vector.bn_stats(out=stats[:, g, :], in_=xr[:, g, :])
                mv = pool.tile([P, 2], FP32, bufs=4, tag=tag + "mv",
                               name=tag + "mv")
                nc.vector.bn_aggr(out=mv[:], in_=stats[:])
                nc.vector.tensor_copy(vb[:, i:i + 1], mv[:, 1:2])
                mvs.append(mv)
            sd = pool.tile([P, 4], FP32, bufs=1, tag=tag + "sd",
                           name=tag + "sd")
            nc.scalar.activation(out=sd[:], in_=vb[:], func=AF.Sqrt,
                                 bias=eps_t[:, 0:1])
            rstd = pool.tile([P, 4], FP32, bufs=1, tag=tag + "rs",
                             name=tag + "rs")
            nc.vector.reciprocal(out=rstd[:], in_=sd[:])
            ys = []
            for i, x_tile in enumerate(x_tiles):
                if apply_ln:
                    yf = pool.tile([P, D], FP32, bufs=2, tag=tag + "yf",
                                   name=tag + "yf")
                    nc.vector.tensor_scalar(out=yf[:], in0=x_tile[:],
                                            scalar1=mvs[i][:, 0:1],
                                            scalar2=rstd[:, i:i + 1],
                                            op0=SUB, op1=MULT)
                    nc.vector.tensor_mul(yf[:], yf[:],
                                         ln_sb[which + "_w"][:])
                    nc.vector.tensor_add(yf[:], yf[:],
                                         ln_sb[which + "_b"][:])
                    y = pool.tile([P, D], BF16, bufs=4, tag=tag + "y",
                                  name=tag + "y")
                    nc.vector.tensor_copy(y[:], yf[:])
                else:
                    y = pool.tile([P, D], BF16, bufs=4, tag=tag + "y",
                                  name=tag + "y")
                    nc.vector.tensor_scalar(out=y[:], in0=x_tile[:],
                                            scalar1=mvs[i][:, 0:1],
                                            scalar2=rstd[:, i:i + 1],
                                            op0=SUB, op1=MULT)
                ys.append(y)
            return ys

        xt = []
        x1 = []
        for st in range(4):
            xti = resid.tile([P, D], FP32, tag=f"xt{st}", name=f"xt{st}")
            xt.append(xti)
            x1t = resid.tile([P, D], FP32, tag=f"x1{st}", name=f"x1_{st}")
            x1.append(x1t)

        # ---------------- P1: LN1 + transpose (dc-major) ----------------
        with tc.tile_pool(name="p1", bufs=1) as p1, \
             tc.tile_pool(name="ps1", bufs=1, space="PSUM") as ps1:
            for st in range(4):
                nc.sync.dma_start(xt[st][:], x_c[st * P:(st + 1) * P, :])
            ys = layer_norm4(p1, xt, "l1", "ln1")
            yT = p1.tile([P, 8, 512], BF16, tag="yT")
            ytc = dram.tile([D, TOK], BF16, tag="ytc")
            for dc in range(8):
                for st in range(4):
                    ptt = ps1.tile([P, P], BF16, bufs=2, tag="pt",
                                   name="ptt")
                    nc.tensor.transpose(ptt[:],
                                        ys[st][:, dc * P:(dc + 1) * P],
                                        ident[:])
                    nc.vector.tensor_copy(
                        yT[:, dc, st * P:(st + 1) * P], ptt[:])
            nc.sync.dma_start(ytc.rearrange("(dc p) t -> p dc t", p=P),
                              yT[:])

        # ---------------- P2: AllGather yT ----------------
        ytg = dram.tile([N_CORES * D, TOK], BF16, tag="ytg")
        nc.gpsimd.collective_compute(
            "AllGather", mybir.AluOpType.bypass, replica_groups=RG,
            ins=[ytc.opt()], outs=[ytg.opt()])

        atc = dram.tile([N_CORES * P, TOK], BF16, tag="atc")

        with tc.tile_pool(name="pwo", bufs=1) as pwo_pool:
            wo_sb = pwo_pool.tile([P, 8, D], BF16, tag="wo")
            nc.sync.dma_start(wo_sb[:],
                              wo_d.rearrange("(dc p) n -> p dc n", p=P))

            with tc.tile_pool(name="p3", bufs=1) as p3:
                masks_sb = p3.tile([P, 4, 512], BF16, tag="masks")
                nc.sync.dma_start(masks_sb[:],
                                  masks_d.rearrange("m p s -> p m s"))
                qTs, kTs = [], []
                for rb in range(8):
                    qt_i = p3.tile([P, 512], BF16, tag=f"qT{rb}",
                                   name=f"qT{rb}")
                    kt_i = p3.tile([P, 512], BF16, tag=f"kT{rb}",
                                   name=f"kT{rb}")
                    qTs.append(qt_i)
                    kTs.append(kt_i)
                vext = p3.tile([P, 32, 130], BF16, tag="vext")

                # ------------ P3: QKV over full sequence ------------
                with tc.tile_pool(name="ps3", bufs=1, space="PSUM") as ps3:
                    for rb in range(8):
                        yts = []
                        for dc in range(8):
                            yt_t = p3.tile([P, 512], BF16, bufs=10,
                                           tag="ytg_t", name="yt_t")
                            base = rb * D + dc * P
                            nc.sync.dma_start(yt_t[:],
                                              ytg[base:base + P, :])
                            yts.append(yt_t)
                        for w_sb, dst in ((wq_sb, qTs[rb]), (wk_sb, kTs[rb])):
                            pq = ps3.tile([P, 512], FP32, bufs=3, tag="pq",
                                          name="pq")
                            for dc in range(8):
                                nc.tensor.matmul(pq[:], w_sb[:, dc, :],
                                                 yts[dc][:],
                                                 start=(dc == 0),
                                                 stop=(dc == 7))
                            nc.scalar.copy(dst[:], pq[:])
                        pv = ps3.tile([P, 512], FP32, bufs=3, tag="pq",
                                      name="pv")
                        for dc in range(8):
                            nc.tensor.matmul(pv[:], wv_sb[:, dc, :],
                                             yts[dc][:], start=(dc == 0),
                                             stop=(dc == 7))
                        vt_tmp = p3.tile([P, 512], BF16, bufs=2, tag="vtt",
                                         name="vt_tmp")
                        nc.scalar.copy(vt_tmp[:], pv[:])
                        for t4 in range(4):
                            tch = rb * 4 + t4
                            pvt = ps3.tile([P, P], BF16, bufs=2, tag="pvt",
                                           name="pvt")
                            nc.tensor.transpose(
                                pvt[:], vt_tmp[:, t4 * P:(t4 + 1) * P],
                                ident[:])
                            nc.vector.tensor_copy(vext[:, tch, 0:64],
                                                  pvt[:, 0:64])
                            nc.vector.tensor_copy(vext[:, tch, 65:129],
                                                  pvt[:, 64:128])
                            nc.vector.tensor_copy(vext[:, tch, 64:65],
                                                  ones_b[:, 0:1])
                            nc.vector.tensor_copy(vext[:, tch, 129:130],
                                                  ones_b[:, 0:1])

                # ------------ P4: attention ------------
                with tc.tile_pool(name="ps4", bufs=1, space="PSUM") as ps4:
                    for b in range(2):
                        for qb in range(4):
                            j_blk = 4 * b + qb
                            qt_blk = qTs[j_blk]
                            nt = 4 * (qb + 1)
                            qz = p3.tile([P, 1024], BF16, bufs=2,
                                         tag="qz", name="qz")
                            nc.vector.tensor_copy(qz[0:64, 0:512],
                                                  qt_blk[0:64, :])
                            nc.vector.tensor_copy(qz[64:128, 0:512],
                                                  zeros_b[64:128, :])
                            nc.vector.tensor_copy(qz[0:64, 512:1024],
                                                  zeros_b[0:64, :])
                            nc.vector.tensor_copy(qz[64:128, 512:1024],
                                                  qt_blk[64:128, :])
                            pts = []
                            for j in range(nt):
                                tch = 16 * b + j
                                kt_blk = kTs[tch // 4]
                                kc = slice((tch % 4) * P,
                                           (tch % 4 + 1) * P)
                                psc0 = ps4.tile([P, 512], FP32, bufs=3,
                                                tag="sc0", name="psc0")
                                psc1 = ps4.tile([P, 512], FP32, bufs=3,
                                                tag="sc1", name="psc1")
                                nc.tensor.matmul(psc0[:], kt_blk[:, kc],
                                                 qz[:, 0:512],
                                                 start=True, stop=True)
                                nc.tensor.matmul(psc1[:], kt_blk[:, kc],
                                                 qz[:, 512:1024],
                                                 start=True, stop=True)
                                pt0 = p3.tile([P, 512], BF16, bufs=20,
                                              tag="pt0", name="pt0")
                                pt1 = p3.tile([P, 512], BF16, bufs=20,
                                              tag="pt1", name="pt1")
                                nc.scalar.activation(out=pt0[:],
                                                     in_=psc0[:],
                                                     func=AF.Exp,
                                                     scale=0.125)
                                nc.scalar.activation(out=pt1[:],
                                                     in_=psc1[:],
                                                     func=AF.Exp,
                                                     scale=0.125)
                                if j >= nt - 4:
                                    m = j - (nt - 4)
                                    nc.vector.tensor_mul(pt0[:], pt0[:],
                                                         masks_sb[:, m, :])
                                    nc.vector.tensor_mul(pt1[:], pt1[:],
                                                         masks_sb[:, m, :])
                                pts.append((tch, pt0, pt1))
                            pa0 = ps4.tile([65, 512], FP32, bufs=1,
                                           tag="pa0", name="pa0")
                            pa1 = ps4.tile([65, 512], FP32, bufs=1,
                                           tag="pa1", name="pa1")
                            for idx, (tch, pt0, pt1) in enumerate(pts):
                                nc.tensor.matmul(pa0[:],
                                                 vext[:, tch, 0:65],
                                                 pt0[:], start=(idx == 0),
                                                 stop=(idx == nt - 1))
                                nc.tensor.matmul(pa1[:],
                                                 vext[:, tch, 65:130],
                                                 pt1[:], start=(idx == 0),
                                                 stop=(idx == nt - 1))

                            for hl, pa in ((0, pa0), (1, pa1)):
                                sa = p3.tile([65, 512], FP32, bufs=2,
                                             tag="sa", name="sa")
                                nc.vector.tensor_copy(sa[:], pa[:])
                                rsf = p3.tile([65, 512], FP32, bufs=2,
                                              tag="rsf", name="rsf")
                                nc.vector.reciprocal(out=rsf[64:65, :],
                                                     in_=sa[64:65, :])
                                rsr = p3.tile([65, 512], BF16, bufs=2,
                                              tag="rsr", name="rsr")
                                nc.vector.tensor_copy(rsr[64:65, :],
                                                      rsf[64:65, :])
                                pb = ps4.tile([64, 512], FP32, bufs=3,
                                              tag="sc0", name="pb")
                                nc.tensor.matmul(pb[:], ones_b[64:65, 0:64],
                                                 rsr[64:65, :],
                                                 start=True, stop=True)
                                an = p3.tile([64, 512], BF16, bufs=2,
                                             tag="an", name="an")
                                nc.vector.tensor_mul(an[:], sa[0:64, :],
                                                     pb[:])
                                row = j_blk * P + hl * 64
                                nc.sync.dma_start(atc[row:row + 64, :],
                                                  an[:])

            # ---------------- P5: AllToAll attnT ----------------
            atg = dram.tile([N_CORES * P, TOK], BF16, tag="atg")
            nc.gpsimd.collective_compute(
                "AllToAll", mybir.AluOpType.bypass, replica_groups=RG,
                ins=[atc.opt()], outs=[atg.opt()])

            # ---------------- P6: wo + residual ----------------
            with tc.tile_pool(name="p6", bufs=1) as p6, \
                 tc.tile_pool(name="ps6", bufs=1, space="PSUM") as ps6:
                at2s = []
                for rr in range(8):
                    a2t = p6.tile([P, 512], BF16, tag=f"at2_{rr}",
                                  name=f"at2_{rr}")
                    nc.sync.dma_start(a2t[:],
                                      atg[rr * P:(rr + 1) * P, :])
                    at2s.append(a2t)
                for st in range(4):
                    for ncol in range(2):
                        pw = ps6.tile([P, 512], FP32, bufs=2, tag="pwo",
                                      name="pw")
                        for rr in range(8):
                            nc.tensor.matmul(
                                pw[:], at2s[rr][:, st * P:(st + 1) * P],
                                wo_sb[:, rr, ncol * 512:(ncol + 1) * 512],
                                start=(rr == 0), stop=(rr == 7))
                        nc.vector.tensor_add(
                            x1[st][:, ncol * 512:(ncol + 1) * 512], pw[:],
                            xt[st][:, ncol * 512:(ncol + 1) * 512])

        # ---------------- P7: LN2 + transpose ----------------
        with tc.tile_pool(name="p7", bufs=1) as p7:
            with tc.tile_pool(name="ps7", bufs=1, space="PSUM") as ps7:
                y2T = p7.tile([P, 8, 512], BF16, tag="y2T")
                y2s = layer_norm4(p7, x1, "l2", "ln2")
                for st in range(4):
                    for dc in range(8):
                        ptt2 = ps7.tile([P, P], BF16, bufs=2, tag="pt2",
                                        name="ptt2")
                        nc.tensor.transpose(ptt2[:],
                                            y2s[st][:, dc * P:(dc + 1) * P],
                                            ident[:])
                        nc.vector.tensor_copy(
                            y2T[:, dc, st * P:(st + 1) * P], ptt2[:])

            # ---------------- P8/P9: FFN ----------------
            with tc.tile_pool(name="p8", bufs=1) as p8, \
                 tc.tile_pool(name="ps8", bufs=1, space="PSUM") as ps8:
                hT = p8.tile([P, 32, 512], BF16, tag="hT")
                for fc in range(32):
                    w1tt = p8.tile([P, D], BF16, bufs=4, tag="w1tt",
                                   name="w1tt")
                    nc.sync.dma_start(w1tt[:], w1t_d[fc, :, :])
                    ph = ps8.tile([P, 512], FP32, bufs=3, tag="ph",
                                  name="ph")
                    for dc in range(8):
                        nc.tensor.matmul(ph[:],
                                         w1tt[:, dc * P:(dc + 1) * P],
                                         y2T[:, dc, :], start=(dc == 0),
                                         stop=(dc == 7))
                    nc.scalar.activation(out=hT[:, fc, :], in_=ph[:],
                                         func=AF.Gelu_apprx_tanh,
                                         bias=b1_sb[:, fc:fc + 1])
                for ncol in range(2):
                    nc2 = slice(ncol * 512, (ncol + 1) * 512)
                    po = [ps8.tile([P, 512], FP32, bufs=1, tag=f"po{sc}",
                                   name=f"po_{ncol}_{sc}")
                          for sc in range(4)]
                    for sc in range(4):
                        nc.tensor.matmul(po[sc][:], ones_b[0:1, :],
                                         b2b[0:1, nc2], start=True,
                                         stop=False)
                    for fc in range(32):
                        w2tt = p8.tile([P, 512], BF16, bufs=4, tag="w2tt",
                                       name="w2tt")
                        nc.sync.dma_start(w2tt[:],
                                          w2t_d[fc * P:(fc + 1) * P, nc2])
                        for sc in range(4):
                            nc.tensor.matmul(
                                po[sc][:], hT[:, fc, sc * P:(sc + 1) * P],
                                w2tt[:], start=False, stop=(fc == 31))
                    for sc in range(4):
                        oh = p8.tile([P, 512], FP32, bufs=2, tag="oh",
                                     name="oh")
                        nc.vector.tensor_add(oh[:], po[sc][:],
                                             x1[sc][:, nc2])
                        nc.sync.dma_start(out_d[sc * P:(sc + 1) * P, nc2],
                                          oh[:])

    nc.compile()
    return nc


def _emit_attn(nc, vext, pa0, pa1, item):
    tch, pt0, pt1, is_first, is_last = item
    nc.tensor.matmul(pa0[:], vext[:, tch, 0:65], pt0[:],
                     start=is_first, stop=is_last)
    nc.tensor.matmul(pa1[:], vext[:, tch, 65:130], pt1[:],
                     start=is_first, stop=is_last)


def _get_nc(apply_ln):
    key = ("nc_v6", apply_ln)
    if key not in _CACHE:
        _CACHE[key] = _build(apply_ln)
    return _CACHE[key]


def _make_masks():
    tt = np.arange(P)[:, None]
    ss = np.arange(512)[None, :]
    return np.stack([(P * m + tt <= ss) for m in range(4)]
                    ).astype(np.float32)


def _bf16(a):
    import ml_dtypes
    return np.asarray(a, dtype=np.float32).astype(ml_dtypes.bfloat16)


def _prepare(inputs):
    x = np.asarray(inputs["x"], dtype=np.float32).reshape(T, D)
    wq = np.asarray(inputs["wq"], dtype=np.float32)
    wk = np.asarray(inputs["wk"], dtype=np.float32)
    wv = np.asarray(inputs["wv"], dtype=np.float32)
    wo = _bf16(inputs["wo"])
    w1 = np.asarray(inputs["w1"], dtype=np.float32)            # [DFF, D]
    # w1tile[fc, p, dc*128+m] = w1[fc*128+m, dc*128+p]
    w1tile = _bf16(np.ascontiguousarray(
        w1.reshape(32, P, 8, P).transpose(0, 3, 2, 1)
        .reshape(32, P, D)))
    w2t = _bf16(np.asarray(inputs["w2"], dtype=np.float32).T)   # [DFF, D]
    b1 = np.asarray(inputs["b1"], dtype=np.float32)
    b2 = np.asarray(inputs["b2"], dtype=np.float32)
    masks = _bf16(_make_masks())

    apply_ln = not (
        np.all(np.asarray(inputs["ln1_w"]) == 1)
        and np.all(np.asarray(inputs["ln1_b"]) == 0)
        and np.all(np.asarray(inputs["ln2_w"]) == 1)
        and np.all(np.asarray(inputs["ln2_b"]) == 0))

    in_maps = []
    for r in range(N_CORES):
        m = {
            "x_c": np.ascontiguousarray(x[r * TOK:(r + 1) * TOK]),
            "wq_c": _bf16(np.concatenate([wq[2 * r], wq[2 * r + 1]],
                                         axis=1)),
            "wk_c": _bf16(np.concatenate([wk[2 * r], wk[2 * r + 1]],
                                         axis=1)),
            "wv_c": _bf16(np.concatenate([wv[2 * r], wv[2 * r + 1]],
                                         axis=1)),
            "wo": wo, "w1tile": w1tile, "w2t": w2t, "b1": b1, "b2": b2,
            "masks": masks,
        }
        if apply_ln:
            for k in ("ln1_w", "ln1_b", "ln2_w", "ln2_b"):
                m[k] = np.asarray(inputs[k], dtype=np.float32)
        in_maps.append(m)
    return in_maps, apply_ln


def _run(inputs, trace=False):
    from concourse.bass_utils import run_bass_kernel_spmd
    in_maps, apply_ln = _prepare(inputs)
    nc = _get_nc(apply_ln)
    res = run_bass_kernel_spmd(nc, in_maps, list(range(N_CORES)),
                               trace=trace)
    out = np.concatenate([res.results[r]["out"] for r in range(N_CORES)],
                         axis=0).reshape(B, S, D).astype(np.float32)
    return out, res


def kernel(**inputs):
    out, _ = _run(inputs)
    return out


def bench(**inputs):
    """Like kernel() but with NTFF tracing; returns (out, exec_time_ns)."""
    out, res = _run(inputs, trace=True)
    return out, res.exec_time_ns
